# revision 10
# baseline (speedup 1.0000x reference)
"""BitNet MLP (act_quant -> ternary matmul -> relu^2 -> SubLN -> act_quant ->
ternary matmul) on 8 Trainium2 NeuronCores, data-parallel over tokens.

Math notes (exactness):
- act_quant int levels (|q| <= 127) and ternary weights {-1,0,1} are exactly
  representable in bf16, so both matmuls run on the PE in bf16 with exact
  integer arithmetic (f32 PSUM accumulation, |sums| < 2^24).
- All quantization scales are folded into per-token scalars applied to the
  final [tok, 512] output: out = i2 * beta_t with
    beta_t = clip(c_t * alpha_t * Sabs_t, 1e-5) * clip(mean|w_dn|,1e-5) / 127
  where alpha_t = (clip(max|x_t|,1e-5) * clip(mean|w_up|,1e-5) / 127)^2,
  Sabs_t = max_i |relu(ih)^2 * g|, c_t = rsqrt(var_t + 1e-6).
- Rounding uses the magic-number trick (x + 1.5*2^23 - 1.5*2^23) == RNE
  round-to-integer for |x| < 2^22, matching jnp.round (half-to-even).
- SubLN variance is recovered from the quantized intermediate:
  var = alpha^2 * sum(iu^2) * (Sabs/127)^2 / (2048 * g0^2); the
  quantization error on sum(iu^2) is ~0.1% which is far below tolerance.
  (For non-constant g an extra pass computes sum((relu^2)^2) directly.)
"""
import os
import numpy as np

import concourse.bass as bass
import concourse.tile as tile
from concourse import mybir
from concourse.bass_utils import run_bass_kernel_spmd
from concourse.masks import make_identity

# ---------------------------------------------------------------------------
# Workaround for walrus "Too many sync wait commands" on the TileContext tail
# drain: split the drain's semaphore waits across single-wait SP NOPs, then
# advance the observed clocks so the real drain needs none.
import re as _re
import bass_rust as _bass_rust


def _patched_drain_and_barrier(self, tick_clock, wait_clock):
    gc = tick_clock.global_clock
    ticks = list(map(int, _re.findall(r"\d+", repr(gc))))
    n = len(ticks)
    nonzero = [(i, t) for i, t in enumerate(ticks) if t > 0]
    for i, t in nonzero:
        sub = [0] * n
        sub[i] = t
        sub_scoped = _bass_rust.ScopedClock({None: _bass_rust.VectorClock(sub)})
        nop = self.nc.sync.nop()
        wait_clock.add_sem_waits(nop.ins, sub_scoped)
        for ec in wait_clock.engine_clocks:
            ec.update_past(sub_scoped)
    drain_inst = self.nc.sync.drain()
    wait_clock.add_sem_waits(drain_inst.ins,
                             _bass_rust.ScopedClock({None: gc}))
    self.nc.all_engine_barrier()
    popped = self.nc._tile_sem_poison_stack.pop()
    assert popped is self._sem_poison
    self.nc.clear_and_free_semaphores(list(self.sems.allocated().values()))
    self.nc.all_engine_barrier()


tile.TileContext._drain_and_barrier = _patched_drain_and_barrier


def _split_sync_waits(nc, keep_default=1):
    """walrus caps the number of semaphore waits a single instruction can
    carry (CTRL ops take only 1; compute ops a few). Hoist excess waits onto
    single-wait NOPs inserted immediately before the instruction on the same
    engine — identical semantics, engines execute in order."""
    import dataclasses
    keep_by_op = {}
    proto = None
    for f in nc.m.functions:
        for bb in f.blocks:
            for inst in bb.instructions:
                if type(inst).__name__ == "InstNoOp":
                    proto = inst
                    break
            if proto is not None:
                break
        if proto is not None:
            break
    counter = [0]
    for f in nc.m.functions:
        new_blocks = []
        for bb in f.blocks:
            out = []
            changed = False
            for inst in bb.instructions:
                si = inst.sync_info
                ow = list(si.on_wait) if si is not None and si.on_wait else []
                keep = keep_by_op.get(inst.opcode, keep_default)
                if len(ow) > keep:
                    assert proto is not None, "no NoOp prototype found yet"
                    for w in ow[:-keep]:
                        counter[0] += 1
                        nop = dataclasses.replace(
                            proto,
                            name=f"I-waitsplit-{counter[0]}",
                            engine=inst.engine,
                            sync_info=_bass_rust.SyncInfo(on_wait=[w],
                                                          on_update=[]),
                        )
                        out.append(nop)
                    si.on_wait = ow[-keep:]
                    changed = True
                out.append(inst)
            if changed:
                bb2 = _bass_rust.BasicBlock(name=bb.name, instructions=out)
                bb2.IsExit = bb.IsExit
                bb2.IsLoopEntry = bb.IsLoopEntry
                bb2.IsPredicated = bb.IsPredicated
                new_blocks.append(bb2)
            else:
                new_blocks.append(bb)
        f.blocks = new_blocks
# ---------------------------------------------------------------------------

F32 = mybir.dt.float32
BF16 = mybir.dt.bfloat16
ALU = mybir.AluOpType
AF = mybir.ActivationFunctionType

N_CORES = 8
B, S, H, I = 8, 8192, 512, 2048
TOK = B * S                  # 65536 tokens total
TPC = TOK // N_CORES         # 8192 tokens per core
P = 128                      # partition tile
NT = TPC // P                # 64 token tiles per core
NKH = H // P                 # 4 k-tiles over H
NKI = I // P                 # 16 k-tiles over I
NB = I // 512                # 4 psum banks for the up matmul

MAGIC = 12582912.0           # 1.5 * 2^23: RNE round-to-int trick
EPS = 1e-6                   # SubLN eps (from reference)

LAST_RESULT = None           # set by kernel() for test harness introspection


def _emit_weight_quant(nc, stage, junkp, ps, consts, wT_dram, n_ktiles,
                       nsub, name, magicb):
    """Quantize a (host-pre-transposed) weight matrix to ternary bf16 tiles.

    wT_dram: [n_ktiles*128, nsub*512] f32 in DRAM (contraction dim on rows).
    Returns (list of [128, nsub, 512] bf16 sbuf tiles, scale_recip [128,1],
    meanclip [128,1]) where meanclip = clip(mean|w|, 1e-5) broadcast to all
    partitions.
    """
    n_elem = n_ktiles * 128 * nsub * 512

    # pass 1: per-partition abs sums
    asum = consts.tile([P, n_ktiles], F32, tag=f"{name}_asum")
    for k in range(n_ktiles):
        wf = stage.tile([P, nsub * 512], F32, tag="stage")
        nc.gpsimd.dma_start(out=wf, in_=wT_dram[k * P:(k + 1) * P, :])
        junk = junkp.tile([P, nsub * 512], BF16, tag="junk")
        nc.scalar.activation(out=junk, in_=wf, func=AF.Abs,
                             accum_out=asum[:, k:k + 1])
    tot = consts.tile([P, 1], F32, tag=f"{name}_tot")
    nc.vector.tensor_reduce(out=tot, in_=asum, axis=mybir.AxisListType.X,
                            op=ALU.add)
    # broadcast-sum across partitions: ones128.T @ tot
    ones128 = stage.tile([P, P], F32, tag="ones128")
    nc.vector.memset(ones128, 1.0)
    totp = ps.tile([P, 1], F32, tag="totp")
    nc.tensor.matmul(out=totp, lhsT=ones128, rhs=tot, start=True, stop=True)
    gsum = consts.tile([P, 1], F32, tag=f"{name}_gsum")
    nc.scalar.copy(out=gsum, in_=totp)
    # mean -> clip -> reciprocal scale
    meanclip = consts.tile([P, 1], F32, tag=f"{name}_meanclip")
    nc.vector.tensor_scalar(out=meanclip, in0=gsum, scalar1=1.0 / n_elem,
                            scalar2=1e-5, op0=ALU.mult, op1=ALU.max)
    swq = consts.tile([P, 1], F32, tag=f"{name}_swq")
    nc.vector.reciprocal(out=swq, in_=meanclip)

    # pass 2: re-load, round+clip to ternary bf16
    wq_tiles = []
    for k in range(n_ktiles):
        wf = stage.tile([P, nsub * 512], F32, tag="stage")
        nc.gpsimd.dma_start(out=wf, in_=wT_dram[k * P:(k + 1) * P, :])
        rt = stage.tile([P, nsub * 512], F32, tag="stage_rt")
        nc.scalar.activation(out=rt, in_=wf, func=AF.Identity,
                             bias=magicb, scale=swq)
        cl = stage.tile([P, nsub * 512], F32, tag="stage_cl")
        nc.vector.tensor_scalar(out=cl, in0=rt, scalar1=MAGIC, scalar2=1.0,
                                op0=ALU.subtract, op1=ALU.min)
        wq = consts.tile([P, nsub, 512], BF16, tag=f"{name}_wq{k}")
        nc.vector.tensor_scalar(out=wq.rearrange("p a b -> p (a b)"), in0=cl,
                                scalar1=-1.0, scalar2=None, op0=ALU.max)
        wq_tiles.append(wq)
    return wq_tiles, meanclip


def build_nc(general_g: bool):
    nc = bass.Bass()
    x_d = nc.dram_tensor("x", [TPC, H], F32, kind="ExternalInput")
    wupT_d = nc.dram_tensor("wupT", [H, I], F32, kind="ExternalInput")
    wdnT_d = nc.dram_tensor("wdnT", [I, H], F32, kind="ExternalInput")
    g_d = nc.dram_tensor("g", [I], F32, kind="ExternalInput")
    out_d = nc.dram_tensor("out", [TPC, H], F32, kind="ExternalOutput")

    from contextlib import ExitStack
    with ExitStack() as ctx:
        tc = ctx.enter_context(tile.TileContext(nc))

        # ---------------- constants / weight prep ----------------
        consts = ctx.enter_context(tc.tile_pool(name="consts", bufs=1))

        ident = consts.tile([P, P], BF16)
        make_identity(nc, ident)

        magicb = consts.tile([P, 1], F32)
        nc.vector.memset(magicb, MAGIC)

        # g broadcast to all partitions: [128, I] f32
        g_bc = consts.tile([P, I], F32)
        g_ap = g_d[:]
        g_bcast_ap = bass.AP(tensor=g_ap.tensor, offset=g_ap.offset,
                             ap=[[0, P]] + list(g_ap.ap))
        nc.gpsimd.dma_start(out=g_bc, in_=g_bcast_ap)

        g0b = consts.tile([P, 1], F32)
        with tc.tile_pool(name="wstage", bufs=2) as stage, \
                tc.tile_pool(name="wjunk", bufs=2) as junkp, \
                tc.tile_pool(name="wps", bufs=1, space="PSUM") as wps:
            # g0 broadcast [128,1] via K=1 matmul with ones
            ones_row = stage.tile([1, P], F32, tag="ones_row")
            nc.vector.memset(ones_row, 1.0)
            g0_sb = stage.tile([1, 1], F32, tag="g0sb")
            nc.gpsimd.dma_start(out=g0_sb, in_=g_d[0:1])
            g0_ps = wps.tile([P, 1], F32, tag="g0ps")
            nc.tensor.matmul(out=g0_ps, lhsT=ones_row, rhs=g0_sb, start=True,
                             stop=True)
            nc.scalar.copy(out=g0b, in_=g0_ps)

            wup_q, up_meanclip = _emit_weight_quant(
                nc, stage, junkp, wps, consts, wupT_d, NKH, NB, "wup", magicb)
            wdn_q, dn_meanclip = _emit_weight_quant(
                nc, stage, junkp, wps, consts, wdnT_d, NKI, 1, "wdn", magicb)

        # k1b = clip(mean|w_up|,1e-5)/127  (per-token gamma multiplier)
        k1b = consts.tile([P, 1], F32)
        nc.vector.tensor_scalar_mul(out=k1b, in0=up_meanclip, scalar1=1.0 / 127.0)
        # wdk = clip(mean|w_dn|,1e-5)/127  (final output multiplier)
        wdk = consts.tile([P, 1], F32)
        nc.vector.tensor_scalar_mul(out=wdk, in0=dn_meanclip, scalar1=1.0 / 127.0)
        # sg127 = sign(g0)*127 (quant scale sign), g0a = |g0|
        sg127 = consts.tile([P, 1], F32)
        nc.scalar.activation(out=sg127, in_=g0b, func=AF.Sign)
        nc.vector.tensor_scalar_mul(out=sg127, in0=sg127, scalar1=127.0)
        g0a = consts.tile([P, 1], F32)
        nc.scalar.activation(out=g0a, in_=g0b, func=AF.Abs)

        # ---------------- main token-tile pipeline ----------------
        # isg = sign(g0)/127 (or 1/127 for general g): folds the quant scale
        # sign so d = recip(clip(S)*isg) = sign*127/clip(S) in 2 small ops.
        isg = consts.tile([P, 1], F32)
        if general_g:
            nc.vector.memset(isg, 1.0 / 127.0)
        else:
            nc.vector.tensor_scalar_mul(out=isg, in0=sg127,
                                        scalar1=1.0 / (127.0 * 127.0))

        BG = 8  # tiles per small-op batch
        KV = 1.0 / (127.0 * 127.0 * I)

        xs_pool = ctx.enter_context(tc.tile_pool(name="xs", bufs=2 * BG))
        xq_pool = ctx.enter_context(tc.tile_pool(name="xqp", bufs=3))
        big = ctx.enter_context(tc.tile_pool(name="big", bufs=2))
        iup = ctx.enter_context(tc.tile_pool(name="iup", bufs=3))
        outp = ctx.enter_context(tc.tile_pool(name="outp", bufs=BG + 1))
        o2p = ctx.enter_context(tc.tile_pool(name="o2p", bufs=3))
        junkp = ctx.enter_context(tc.tile_pool(name="mjunk", bufs=1))
        small = ctx.enter_context(tc.tile_pool(name="small", bufs=3))
        batchp = ctx.enter_context(tc.tile_pool(name="batchp", bufs=2))
        ps_xT = ctx.enter_context(tc.tile_pool(name="ps_xT", bufs=1,
                                               space="PSUM"))
        ps_ih = ctx.enter_context(tc.tile_pool(name="ps_ih", bufs=1,
                                               space="PSUM"))
        ps_iuT = ctx.enter_context(tc.tile_pool(name="ps_iuT", bufs=1,
                                                space="PSUM"))
        ps_o = ctx.enter_context(tc.tile_pool(name="ps_o", bufs=1,
                                              space="PSUM"))

        IH2 = I // 2  # up-matmul accumulates in two 2-bank halves

        def phase_a(ib, state):
            """DMA x tiles + per-token absmax, then batched x-scale chain."""
            xm8 = batchp.tile([P, BG], F32, tag="xm8")
            x_tiles = []
            for j in range(BG):
                r0 = (ib + j) * P
                x_sb = xs_pool.tile([P, H], F32, tag="x")
                nc.sync.dma_start(out=x_sb, in_=x_d[r0:r0 + P, :])
                x_tiles.append(x_sb)
                nc.vector.tensor_reduce(out=xm8[:, j:j + 1], in_=x_sb,
                                        axis=mybir.AxisListType.X, op=ALU.max,
                                        apply_absolute_value=True)
            t08 = batchp.tile([P, BG], F32, tag="t08")
            nc.vector.tensor_scalar_max(out=t08, in0=xm8, scalar1=1e-5)
            xr8 = batchp.tile([P, BG], F32, tag="xr8")
            nc.vector.reciprocal(out=xr8, in_=t08)
            xsc8 = batchp.tile([P, BG], F32, tag="xsc8")
            nc.vector.tensor_scalar_mul(out=xsc8, in0=xr8, scalar1=127.0)
            state[ib] = (x_tiles, t08, xsc8)

        def phase_bc(ib, state):
            x_tiles, t08, xsc8 = state.pop(ib)
            Sm8 = batchp.tile([P, BG], F32, tag="Sm8")
            q28 = batchp.tile([P, BG], F32, tag="q28")
            q2g8 = None
            if general_g:
                q2g8 = batchp.tile([P, BG], F32, tag="q2g8")
            o_tiles = []

            for j in range(BG):
                x_sb = x_tiles[j]
                # quantize x (RNE round via magic): ACT + DVE
                xq = xq_pool.tile([P, H], F32, tag="xq")
                nc.scalar.activation(out=xq, in_=x_sb, func=AF.Identity,
                                     bias=magicb, scale=xsc8[:, j:j + 1])
                ix = xq_pool.tile([P, H], BF16, tag="ix")
                nc.vector.tensor_scalar(out=ix, in0=xq, scalar1=MAGIC,
                                        scalar2=None, op0=ALU.subtract)
                # transpose ix via PE, drain on ACT
                xT_ps = ps_xT.tile([P, NKH, P], BF16, tag="xT")
                for k in range(NKH):
                    nc.tensor.transpose(out=xT_ps[:, k, :],
                                        in_=ix[:, k * P:(k + 1) * P],
                                        identity=ident)
                xT_sb = xq_pool.tile([P, NKH, P], BF16, tag="xTsb")
                nc.scalar.copy(out=xT_sb, in_=xT_ps)

                # up matmul in two halves (each 2 PSUM banks) so the next
                # tile's matmuls only wait on a half-drain
                r_sb = big.tile([P, I], F32, tag="r")
                smh = small.tile([P, 2], F32, tag="smh")
                for h in range(2):
                    ihh = ps_ih.tile([P, IH2], F32, tag="ih")
                    for nb in range(2):
                        lo = nb * 512
                        for k in range(NKH):
                            nc.tensor.matmul(
                                out=ihh[:, lo:lo + 512],
                                lhsT=xT_sb[:, k, :],
                                rhs=wup_q[k][:, 2 * h + nb, :],
                                start=(k == 0), stop=(k == NKH - 1))
                    nc.scalar.activation(out=r_sb[:, h * IH2:(h + 1) * IH2],
                                         in_=ihh, func=AF.Relu)
                    if not general_g:
                        nc.vector.tensor_reduce(out=smh[:, h:h + 1], in_=ihh,
                                                axis=mybir.AxisListType.X,
                                                op=ALU.max)

                if general_g:
                    s_sb = big.tile([P, I], F32, tag="s")
                    nc.gpsimd.tensor_tensor(out=s_sb, in0=r_sb, in1=r_sb,
                                            op=ALU.mult)
                    sq_in = big.tile([P, I], F32, tag="sg")
                    nc.vector.tensor_tensor(out=sq_in, in0=s_sb, in1=g_bc,
                                            op=ALU.mult)
                    junk3 = junkp.tile([P, I], BF16, tag="junk3")
                    nc.scalar.activation(out=junk3, in_=s_sb, func=AF.Square,
                                         accum_out=q2g8[:, j:j + 1])
                    nc.vector.tensor_reduce(out=Sm8[:, j:j + 1], in_=sq_in,
                                            axis=mybir.AxisListType.X,
                                            op=ALU.max,
                                            apply_absolute_value=True)
                    sc2 = small.tile([P, 1], F32, tag="sc2")
                    nc.vector.tensor_scalar(out=sc2, in0=Sm8[:, j:j + 1],
                                            scalar1=1e-30, scalar2=isg,
                                            op0=ALU.max, op1=ALU.mult)
                    dr = small.tile([P, 1], F32, tag="dr")
                    nc.vector.reciprocal(out=dr, in_=sc2)
                    rt = big.tile([P, I], F32, tag="rt")
                    nc.vector.tensor_scalar(out=rt, in0=sq_in, scalar1=dr,
                                            scalar2=MAGIC, op0=ALU.mult,
                                            op1=ALU.add)
                    iu = iup.tile([P, I], BF16, tag="iu")
                    nc.vector.tensor_scalar(out=iu, in0=rt, scalar1=MAGIC,
                                            scalar2=None, op0=ALU.subtract)
                else:
                    # s' = r*r on GPSIMD
                    s_sb = big.tile([P, I], F32, tag="s")
                    nc.gpsimd.tensor_tensor(out=s_sb, in0=r_sb, in1=r_sb,
                                            op=ALU.mult)
                    # rmax = max over both halves (straight from PSUM above)
                    nc.vector.tensor_reduce(out=Sm8[:, j:j + 1], in_=smh,
                                            axis=mybir.AxisListType.X,
                                            op=ALU.max)
                    mr = small.tile([P, 1], F32, tag="mr")
                    nc.vector.tensor_scalar_max(out=mr, in0=Sm8[:, j:j + 1],
                                                scalar1=1e-15)
                    sc2 = small.tile([P, 1], F32, tag="sc2")
                    nc.vector.tensor_scalar(out=sc2, in0=mr, scalar1=mr,
                                            scalar2=isg, op0=ALU.mult,
                                            op1=ALU.mult)
                    dr = small.tile([P, 1], F32, tag="dr")
                    nc.vector.reciprocal(out=dr, in_=sc2)
                    rt = big.tile([P, I], F32, tag="rt")
                    nc.vector.tensor_scalar(out=rt, in0=s_sb, scalar1=dr,
                                            scalar2=MAGIC, op0=ALU.mult,
                                            op1=ALU.add)
                    iu = iup.tile([P, I], BF16, tag="iu")
                    nc.vector.tensor_scalar(out=iu, in0=rt, scalar1=MAGIC,
                                            scalar2=None, op0=ALU.subtract)

                # q2 = sum(iu^2) (ACT square + accumulate)
                junk2 = junkp.tile([P, I], BF16, tag="junk2")
                nc.scalar.activation(out=junk2, in_=iu, func=AF.Square,
                                     accum_out=q28[:, j:j + 1])

                # transpose iu via PE, drain on ACT
                iuT_ps = ps_iuT.tile([P, NKI, P], BF16, tag="iuT")
                for k in range(NKI):
                    nc.tensor.transpose(out=iuT_ps[:, k, :],
                                        in_=iu[:, k * P:(k + 1) * P],
                                        identity=ident)
                iuT_sb = iup.tile([P, NKI, P], BF16, tag="iuTsb")
                nc.scalar.copy(out=iuT_sb, in_=iuT_ps)

                # down matmul + plain drain (beta applied later, batched)
                o_ps = ps_o.tile([P, H], F32, tag="o")
                for k in range(NKI):
                    nc.tensor.matmul(out=o_ps, lhsT=iuT_sb[:, k, :],
                                     rhs=wdn_q[k][:, 0, :],
                                     start=(k == 0), stop=(k == NKI - 1))
                o_sb = outp.tile([P, H], F32, tag="osb")
                nc.scalar.copy(out=o_sb, in_=o_ps)
                o_tiles.append(o_sb)

            # --- batched beta chain ---
            scc8 = batchp.tile([P, BG], F32, tag="scc8")
            if general_g:
                nc.vector.tensor_scalar_max(out=scc8, in0=Sm8, scalar1=1e-30)
            else:
                ra8 = batchp.tile([P, BG], F32, tag="ra8")
                nc.vector.tensor_scalar_max(out=ra8, in0=Sm8, scalar1=0.0)
                ssq8 = batchp.tile([P, BG], F32, tag="ssq8")
                nc.vector.tensor_tensor(out=ssq8, in0=ra8, in1=ra8,
                                        op=ALU.mult)
                nc.vector.tensor_scalar_max(out=scc8, in0=ssq8,
                                            scalar1=1e-30)
            ga8 = batchp.tile([P, BG], F32, tag="ga8")
            nc.vector.tensor_scalar_mul(out=ga8, in0=t08, scalar1=k1b)
            al8 = batchp.tile([P, BG], F32, tag="al8")
            nc.vector.tensor_tensor(out=al8, in0=ga8, in1=ga8, op=ALU.mult)
            m18 = batchp.tile([P, BG], F32, tag="m18")
            nc.vector.tensor_tensor(out=m18, in0=al8, in1=scc8, op=ALU.mult)
            v18 = batchp.tile([P, BG], F32, tag="v18")
            Ve8 = batchp.tile([P, BG], F32, tag="Ve8")
            if general_g:
                al28 = batchp.tile([P, BG], F32, tag="al28")
                nc.vector.tensor_tensor(out=al28, in0=al8, in1=al8,
                                        op=ALU.mult)
                nc.vector.tensor_tensor(out=v18, in0=al28, in1=q2g8,
                                        op=ALU.mult)
                nc.vector.tensor_scalar(out=Ve8, in0=v18, scalar1=1.0 / I,
                                        scalar2=EPS, op0=ALU.mult,
                                        op1=ALU.add)
            else:
                m28 = batchp.tile([P, BG], F32, tag="m28")
                nc.vector.tensor_tensor(out=m28, in0=m18, in1=m18,
                                        op=ALU.mult)
                nc.vector.tensor_tensor(out=v18, in0=m28, in1=q28,
                                        op=ALU.mult)
                nc.vector.tensor_scalar(out=Ve8, in0=v18, scalar1=KV,
                                        scalar2=EPS, op0=ALU.mult,
                                        op1=ALU.add)
            sq8 = batchp.tile([P, BG], F32, tag="sq8")
            nc.scalar.activation(out=sq8, in_=Ve8, func=AF.Sqrt)
            cr8 = batchp.tile([P, BG], F32, tag="cr8")
            nc.vector.reciprocal(out=cr8, in_=sq8)
            h18 = batchp.tile([P, BG], F32, tag="h18")
            nc.vector.tensor_tensor(out=h18, in0=cr8, in1=cr8, op=ALU.mult)
            h28 = batchp.tile([P, BG], F32, tag="h28")
            nc.vector.tensor_tensor(out=h28, in0=h18, in1=Ve8, op=ALU.mult)
            h38 = batchp.tile([P, BG], F32, tag="h38")
            nc.vector.tensor_scalar(out=h38, in0=h28, scalar1=-0.5,
                                    scalar2=1.5, op0=ALU.mult, op1=ALU.add)
            c8 = batchp.tile([P, BG], F32, tag="c8")
            nc.vector.tensor_tensor(out=c8, in0=cr8, in1=h38, op=ALU.mult)
            if general_g:
                m1g8 = m18
            else:
                m1g8 = batchp.tile([P, BG], F32, tag="m1g8")
                nc.vector.tensor_scalar_mul(out=m1g8, in0=m18, scalar1=g0a)
            mu8 = batchp.tile([P, BG], F32, tag="mu8")
            nc.vector.tensor_tensor(out=mu8, in0=c8, in1=m1g8, op=ALU.mult)
            b8 = batchp.tile([P, BG], F32, tag="b8")
            nc.vector.tensor_scalar(out=b8, in0=mu8, scalar1=1e-5,
                                    scalar2=wdk, op0=ALU.max, op1=ALU.mult)

            # --- scale + store ---
            for j in range(BG):
                r0 = (ib + j) * P
                o2 = o2p.tile([P, H], F32, tag="o2")
                nc.vector.tensor_scalar_mul(out=o2, in0=o_tiles[j],
                                            scalar1=b8[:, j:j + 1])
                nc.sync.dma_start(out=out_d[r0:r0 + P, :], in_=o2)

        # software-pipelined emission: batch ib+1's loads are issued before
        # batch ib's compute so DMA/absmax overlap the previous batch
        state = {}
        phase_a(0, state)
        for ib in range(0, NT, BG):
            if ib + BG < NT:
                phase_a(ib + BG, state)
            phase_bc(ib, state)

    _split_sync_waits(nc)
    return nc


def build_fast(g0: float):
    """Const-g fast path.

    Per 128-token tile (exact integer math, scales folded into final beta):
      DVE : xq = x*sc + MAGIC           (round-to-int via magic, f32)
      ACT : ix = xq - MAGIC -> bf16     (exact int8 levels)
      PE  : xT = transpose(ix)          (4x N=128)
      PE  : h  = ixT.T @ wup_q          (16x N=512 bf16, exact ints in PSUM)
      DVE : s2 = max(h,0)*h             (= relu(h)^2, one STT from PSUM)
      DVE : S2m = max(s2)               (= Rm^2, feeds d = 127/Rm^2)
      ACT : t1 = s2*d + MAGIC           (per-token scale via ACT scale port)
      DVE : iu = t1 - MAGIC -> bf16     (exact int levels 0..127)
      GPS : q2 = sum(iu^2)              (STT with accum, junk main output)
      PE  : iuT = transpose(iu)         (16x N=128), ACT drains
      PE  : o  = iuT.T @ wdn_q          (16x N=512)
      ACT : o_sb = copy(o)
      DVE : out = o_sb * beta, DMA out  (beta via batched per-8 chain)
    Down-matmuls are emitted one 4-tile group behind the up-matmuls so the
    PE never waits on the s2->iu chain.
    """
    nc = bass.Bass()
    x_d = nc.dram_tensor("x", [TPC, H], F32, kind="ExternalInput")
    wupT_d = nc.dram_tensor("wupT", [H, I], F32, kind="ExternalInput")
    wdnT_d = nc.dram_tensor("wdnT", [I, H], F32, kind="ExternalInput")
    out_d = nc.dram_tensor("out", [TPC, H], F32, kind="ExternalOutput")

    BG = 8          # stats/beta batch
    GRP = 4         # pipeline group (down-matmul lag)
    IH2 = I // 2
    KV = 1.0 / (127.0 * 127.0 * I)
    g0a = abs(g0)
    g0s = 1.0 if g0 >= 0 else -1.0

    from contextlib import ExitStack
    with ExitStack() as ctx:
        tc = ctx.enter_context(tile.TileContext(nc))

        consts = ctx.enter_context(tc.tile_pool(name="consts", bufs=1))
        ident = consts.tile([P, P], BF16)
        make_identity(nc, ident)
        identf = consts.tile([P, P], F32)
        make_identity(nc, identf)
        wup_q = consts.tile([P, NKH, I], BF16)
        wdn_q = consts.tile([P, NKI, H], BF16)
        k1b = consts.tile([P, 1], F32)
        wdk = consts.tile([P, 1], F32)
        magicb = consts.tile([P, 1], F32)
        nc.vector.memset(magicb, MAGIC)
        nmagicb = consts.tile([P, 1], F32)
        nc.vector.memset(nmagicb, -MAGIC)

        # pools that must exist before weight prep so x loads / absmax /
        # quant / transposes overlap the prologue
        xs_pool = ctx.enter_context(tc.tile_pool(name="xs", bufs=10))
        batchp = ctx.enter_context(tc.tile_pool(name="batchp", bufs=3))
        xq_pool = ctx.enter_context(tc.tile_pool(name="xqp", bufs=2))
        ix_pool = ctx.enter_context(tc.tile_pool(name="ixp", bufs=2))
        xT_pool = ctx.enter_context(tc.tile_pool(name="xTp", bufs=2))
        ps_tp = ctx.enter_context(tc.tile_pool(name="ps_tp", bufs=2,
                                               space="PSUM"))

        state = {}

        def load_batch(ib):
            xm8 = batchp.tile([P, BG], F32, tag="xm8")
            x_tiles = []
            for jj in range(BG):
                r0 = (ib + jj) * P
                x_sb = xs_pool.tile([P, H], F32, tag="x")
                nc.sync.dma_start(out=x_sb, in_=x_d[r0:r0 + P, :])
                nc.vector.tensor_reduce(out=xm8[:, jj:jj + 1], in_=x_sb,
                                        axis=mybir.AxisListType.X, op=ALU.max,
                                        apply_absolute_value=True)
                x_tiles.append(x_sb)
            t08 = batchp.tile([P, BG], F32, tag="t08")
            nc.vector.tensor_scalar_max(out=t08, in0=xm8, scalar1=1e-5)
            xr8 = batchp.tile([P, BG], F32, tag="xr8")
            nc.vector.reciprocal(out=xr8, in_=t08)
            xsc8 = batchp.tile([P, BG], F32, tag="xsc8")
            nc.vector.tensor_scalar_mul(out=xsc8, in0=xr8, scalar1=127.0)
            S2m8 = batchp.tile([P, BG], F32, tag="S2m8")
            q28 = batchp.tile([P, BG], F32, tag="q28")
            state[ib] = dict(x=x_tiles, t08=t08, xsc8=xsc8, S2m8=S2m8,
                             q28=q28)

        load_batch(0)
        load_batch(BG)

        # ---------------- weight prep (single load) ----------------
        with tc.tile_pool(name="wstage", bufs=1) as wst, \
                tc.tile_pool(name="wscr", bufs=2) as wsc, \
                tc.tile_pool(name="wjunk", bufs=2) as wjk, \
                tc.tile_pool(name="wps", bufs=1, space="PSUM") as wps:
            up_st = wst.tile([P, NKH, I], F32, tag="upst")
            dn_st = wst.tile([P, NKI, H], F32, tag="dnst")
            for k in range(NKH):
                nc.gpsimd.dma_start(out=up_st[:, k, :],
                                    in_=wupT_d[k * P:(k + 1) * P, :])

            asum_u = consts.tile([P, NKH], F32)
            asum_d = consts.tile([P, NKH], F32)
            for k in range(NKH):
                jku = wjk.tile([P, I], BF16, tag="jk")
                nc.scalar.activation(out=jku, in_=up_st[:, k, :], func=AF.Abs,
                                     accum_out=asum_u[:, k:k + 1])
            ones128 = wsc.tile([P, P], F32, tag="ones", bufs=1)
            nc.vector.memset(ones128, 1.0)

            def total_meanclip(asum, n_elem, mc_out):
                tot = consts.tile([P, 1], F32)
                nc.vector.tensor_reduce(out=tot, in_=asum,
                                        axis=mybir.AxisListType.X, op=ALU.add)
                totp = wps.tile([P, 1], F32, tag="totp")
                nc.tensor.matmul(out=totp, lhsT=ones128, rhs=tot, start=True,
                                 stop=True)
                gsum = consts.tile([P, 1], F32)
                nc.scalar.copy(out=gsum, in_=totp)
                nc.vector.tensor_scalar(out=mc_out, in0=gsum,
                                        scalar1=1.0 / n_elem, scalar2=1e-5,
                                        op0=ALU.mult, op1=ALU.max)

            mc_u = consts.tile([P, 1], F32)
            mc_d = consts.tile([P, 1], F32)
            total_meanclip(asum_u, H * I, mc_u)
            swq_u = consts.tile([P, 1], F32)
            nc.vector.reciprocal(out=swq_u, in_=mc_u)
            nc.vector.tensor_scalar_mul(out=k1b, in0=mc_u, scalar1=1.0 / 127.0)

            def quant_chunk(src, dst, swq):
                qt = wsc.tile(list(src.shape), F32, tag="qt")
                nc.scalar.activation(out=qt, in_=src, func=AF.Identity,
                                     bias=magicb, scale=swq)
                qu = wsc.tile(list(src.shape), F32, tag="qu")
                nc.vector.tensor_scalar(out=qu, in0=qt, scalar1=MAGIC,
                                        scalar2=1.0, op0=ALU.subtract,
                                        op1=ALU.min)
                nc.vector.tensor_scalar(out=dst, in0=qu, scalar1=-1.0,
                                        scalar2=None, op0=ALU.max)

            for k in range(NKH):
                quant_chunk(up_st[:, k, :], wup_q[:, k, :], swq_u)

            # --- wdn prep after wup so the first up-matmuls start earlier
            for k in range(NKI):
                nc.gpsimd.dma_start(out=dn_st[:, k, :],
                                    in_=wdnT_d[k * P:(k + 1) * P, :])
            for k in range(NKH):
                jkd = wjk.tile([P, NKH, H], BF16, tag="jkd")
                nc.scalar.activation(out=jkd,
                                     in_=dn_st[:, k * NKH:(k + 1) * NKH, :],
                                     func=AF.Abs,
                                     accum_out=asum_d[:, k:k + 1])
            total_meanclip(asum_d, H * I, mc_d)
            swq_d = consts.tile([P, 1], F32)
            nc.vector.reciprocal(out=swq_d, in_=mc_d)
            nc.vector.tensor_scalar_mul(out=wdk, in0=mc_d, scalar1=1.0 / 127.0)
            for k in range(NKH):
                quant_chunk(dn_st[:, k * NKH:(k + 1) * NKH, :],
                            wdn_q[:, k * NKH:(k + 1) * NKH, :], swq_d)

        # ---------------- main-loop pools ----------------
        r_pool = ctx.enter_context(tc.tile_pool(name="rp", bufs=3))
        s2_pool = ctx.enter_context(tc.tile_pool(name="s2p", bufs=2))
        t1_pool = ctx.enter_context(tc.tile_pool(name="t1p", bufs=2))
        iu_pool = ctx.enter_context(tc.tile_pool(name="iup", bufs=9))
        iuT_pool = ctx.enter_context(tc.tile_pool(name="iuTp", bufs=5))
        dj_pool = ctx.enter_context(tc.tile_pool(name="djp", bufs=2))
        o_pool = ctx.enter_context(tc.tile_pool(name="op", bufs=10))
        o2_pool = ctx.enter_context(tc.tile_pool(name="o2p", bufs=4))
        d_pool = ctx.enter_context(tc.tile_pool(name="dp", bufs=2))
        bb_pool = ctx.enter_context(tc.tile_pool(name="bbp", bufs=2))
        ps_h = ctx.enter_context(tc.tile_pool(name="ps_h", bufs=2,
                                              space="PSUM"))
        ps_o = ctx.enter_context(tc.tile_pool(name="ps_o", bufs=2,
                                              space="PSUM"))

        tstate = {}

        def front(j):
            ib = (j // BG) * BG
            jj = j - ib
            st = state[ib]
            x_sb = st["x"][jj]
            xq = xq_pool.tile([P, H], F32, tag="xq")
            nc.vector.tensor_scalar(out=xq, in0=x_sb,
                                    scalar1=st["xsc8"][:, jj:jj + 1],
                                    scalar2=MAGIC, op0=ALU.mult, op1=ALU.add)
            ixt = ix_pool.tile([P, H], BF16, tag="ix")
            nc.scalar.activation(out=ixt, in_=xq, func=AF.Identity,
                                 bias=nmagicb)
            tp = ps_tp.tile([P, 8, P], BF16, tag="tp")
            for k in range(NKH):
                nc.tensor.transpose(out=tp[:, k, :],
                                    in_=ixt[:, k * P:(k + 1) * P],
                                    identity=ident)
            xT = xT_pool.tile([P, NKH, P], BF16, tag="xT")
            nc.scalar.copy(out=xT, in_=tp[:, :NKH, :])
            r = r_pool.tile([P, I], F32, tag="r")
            for half in range(2):
                hh = ps_h.tile([P, IH2], F32, tag="h")
                for nb in range(2):
                    lo = nb * 512
                    for k in range(NKH):
                        nc.tensor.matmul(
                            out=hh[:, lo:lo + 512],
                            lhsT=xT[:, k, :],
                            rhs=wup_q[:, k, (2 * half + nb) * 512:
                                      (2 * half + nb + 1) * 512],
                            start=(k == 0), stop=(k == NKH - 1))
                nc.scalar.activation(out=r[:, half * IH2:(half + 1) * IH2],
                                     in_=hh, func=AF.Relu)
            # per-token Rm = max(relu(h)) (>= 0 since r >= 0)
            nc.vector.tensor_reduce(out=st["S2m8"][:, jj:jj + 1], in_=r,
                                    axis=mybir.AxisListType.X, op=ALU.max)
            # s2 = relu(h)^2 off the critical DVE/ACT paths
            s2 = s2_pool.tile([P, I], F32, tag="s2")
            nc.gpsimd.tensor_tensor(out=s2, in0=r, in1=r, op=ALU.mult)
            tstate[j] = dict(s2=s2)

        def dbatch(b):
            ib = (b // BG) * BG
            jj0 = b - ib
            S2m8 = state[ib]["S2m8"]
            mr4 = d_pool.tile([P, GRP], F32, tag="mr4")
            nc.vector.tensor_scalar_max(out=mr4, in0=S2m8[:, jj0:jj0 + GRP],
                                        scalar1=1e-15)
            sc4 = d_pool.tile([P, GRP], F32, tag="sc4")
            nc.vector.tensor_tensor(out=sc4, in0=mr4, in1=mr4, op=ALU.mult)
            sc4b = d_pool.tile([P, GRP], F32, tag="sc4b")
            nc.vector.tensor_scalar_mul(out=sc4b, in0=sc4,
                                        scalar1=1.0 / 127.0)
            d4 = d_pool.tile([P, GRP], F32, tag="d4")
            nc.vector.reciprocal(out=d4, in_=sc4b)
            for j in range(b, b + GRP):
                tstate[j]["d"] = d4[:, j - b:j - b + 1]

        def quant_a(j):
            ts = tstate[j]
            t1 = t1_pool.tile([P, I], F32, tag="t1")
            nc.scalar.activation(out=t1, in_=ts["s2"], func=AF.Identity,
                                 bias=magicb, scale=ts["d"])
            iu = iu_pool.tile([P, I], BF16, tag="iu")
            nc.vector.tensor_scalar(out=iu, in0=t1, scalar1=MAGIC,
                                    scalar2=None, op0=ALU.subtract)
            ts["iu"] = iu
            del ts["s2"]

        def quant_b(j):
            ts = tstate[j]
            iu = ts.pop("iu")
            iuT = iuT_pool.tile([P, NKI, P], BF16, tag="iuT")
            for half in range(2):
                tp2 = ps_tp.tile([P, 8, P], BF16, tag="tp")
                for c in range(8):
                    kk = half * 8 + c
                    nc.tensor.transpose(out=tp2[:, c, :],
                                        in_=iu[:, kk * P:(kk + 1) * P],
                                        identity=ident)
                nc.scalar.copy(out=iuT[:, half * 8:(half + 1) * 8, :],
                               in_=tp2)
            ts["iuT"] = iuT

        def down(j):
            ib = (j // BG) * BG
            jj = j - ib
            ts = tstate[j]
            o_ps = ps_o.tile([P, H], F32, tag="o")
            for k in range(NKI):
                nc.tensor.matmul(out=o_ps, lhsT=ts["iuT"][:, k, :],
                                 rhs=wdn_q[:, k, :],
                                 start=(k == 0), stop=(k == NKI - 1))
            o_sb = o_pool.tile([P, H], F32, tag="o_sb")
            nc.scalar.copy(out=o_sb, in_=o_ps)
            # q2 = sum(iu^2) via the diagonal of iuT.T @ iuT on the PE
            dg_ps = ps_tp.tile([P, P], F32, tag="tp")
            for k in range(NKI):
                nc.tensor.matmul(out=dg_ps, lhsT=ts["iuT"][:, k, :],
                                 rhs=ts["iuT"][:, k, :],
                                 start=(k == 0), stop=(k == NKI - 1))
            dj = dj_pool.tile([P, P], F32, tag="dj")
            nc.vector.tensor_tensor(out=dj, in0=dg_ps, in1=identf,
                                    op=ALU.mult)
            nc.vector.tensor_reduce(out=state[ib]["q28"][:, jj:jj + 1],
                                    in_=dj, axis=mybir.AxisListType.X,
                                    op=ALU.add)
            ts["o"] = o_sb
            del ts["iuT"]

        def bbatch(ib):
            st = state[ib]
            ga8 = bb_pool.tile([P, BG], F32, tag="ga8")
            nc.vector.tensor_scalar_mul(out=ga8, in0=st["t08"], scalar1=k1b)
            al8 = bb_pool.tile([P, BG], F32, tag="al8")
            nc.vector.tensor_tensor(out=al8, in0=ga8, in1=ga8, op=ALU.mult)
            ssq8 = bb_pool.tile([P, BG], F32, tag="ssq8")
            nc.vector.tensor_tensor(out=ssq8, in0=st["S2m8"], in1=st["S2m8"],
                                    op=ALU.mult)
            scc8 = bb_pool.tile([P, BG], F32, tag="scc8")
            nc.vector.tensor_scalar_max(out=scc8, in0=ssq8, scalar1=1e-30)
            m18 = bb_pool.tile([P, BG], F32, tag="m18")
            nc.vector.tensor_tensor(out=m18, in0=al8, in1=scc8, op=ALU.mult)
            m28 = bb_pool.tile([P, BG], F32, tag="m28")
            nc.vector.tensor_tensor(out=m28, in0=m18, in1=m18, op=ALU.mult)
            v18 = bb_pool.tile([P, BG], F32, tag="v18")
            nc.vector.tensor_tensor(out=v18, in0=m28, in1=st["q28"],
                                    op=ALU.mult)
            Ve8 = bb_pool.tile([P, BG], F32, tag="Ve8")
            nc.vector.tensor_scalar(out=Ve8, in0=v18, scalar1=KV,
                                    scalar2=EPS, op0=ALU.mult, op1=ALU.add)
            sq8 = bb_pool.tile([P, BG], F32, tag="sq8")
            nc.scalar.activation(out=sq8, in_=Ve8, func=AF.Sqrt)
            cr8 = bb_pool.tile([P, BG], F32, tag="cr8")
            nc.vector.reciprocal(out=cr8, in_=sq8)
            h18 = bb_pool.tile([P, BG], F32, tag="h18")
            nc.vector.tensor_tensor(out=h18, in0=cr8, in1=cr8, op=ALU.mult)
            h28 = bb_pool.tile([P, BG], F32, tag="h28")
            nc.vector.tensor_tensor(out=h28, in0=h18, in1=Ve8, op=ALU.mult)
            h38 = bb_pool.tile([P, BG], F32, tag="h38")
            nc.vector.tensor_scalar(out=h38, in0=h28, scalar1=-0.5,
                                    scalar2=1.5, op0=ALU.mult, op1=ALU.add)
            c8 = bb_pool.tile([P, BG], F32, tag="c8")
            nc.vector.tensor_tensor(out=c8, in0=cr8, in1=h38, op=ALU.mult)
            mu8 = bb_pool.tile([P, BG], F32, tag="mu8")
            nc.vector.tensor_tensor(out=mu8, in0=c8, in1=m18, op=ALU.mult)
            mc8 = bb_pool.tile([P, BG], F32, tag="mc8")
            nc.vector.tensor_scalar(out=mc8, in0=mu8, scalar1=g0a,
                                    scalar2=1e-5, op0=ALU.mult, op1=ALU.max)
            b8 = bb_pool.tile([P, BG], F32, tag="b8")
            nc.vector.tensor_scalar(out=b8, in0=mc8, scalar1=wdk,
                                    scalar2=g0s, op0=ALU.mult, op1=ALU.mult)
            st["b8"] = b8

        def outt(j):
            ib = (j // BG) * BG
            jj = j - ib
            ts = tstate.pop(j)
            b8 = state[ib]["b8"]
            o2 = o2_pool.tile([P, H], F32, tag="o2")
            nc.vector.tensor_scalar_mul(out=o2, in0=ts["o"],
                                        scalar1=b8[:, jj:jj + 1])
            nc.sync.dma_start(out=out_d[j * P:(j + 1) * P, :], in_=o2)

        for b in range(0, NT, GRP):
            if b % BG == 0 and b >= BG and b + BG < NT:
                load_batch(b + BG)
            for j in range(b, b + GRP):
                front(j)
            dbatch(b)
            for j in range(b, b + GRP):
                quant_a(j)
            if b >= GRP:
                for j in range(b - GRP, b):
                    quant_b(j)
                for j in range(b - GRP, b):
                    down(j)
            if b % BG == 0 and b >= 2 * GRP:
                bbatch(b - 2 * GRP)
            if b >= 2 * GRP:
                for j in range(b - 2 * GRP, b - GRP):
                    outt(j)
        for j in range(NT - GRP, NT):
            quant_b(j)
        for j in range(NT - GRP, NT):
            down(j)
        bbatch(NT - 2 * GRP)
        for j in range(NT - 2 * GRP, NT):
            outt(j)

    _split_sync_waits(nc)
    return nc


_NC_CACHE = {}


def kernel(x, w_up, w_down, g):
    global LAST_RESULT
    x = np.ascontiguousarray(x, dtype=np.float32)
    w_up = np.ascontiguousarray(w_up, dtype=np.float32)
    w_down = np.ascontiguousarray(w_down, dtype=np.float32)
    g = np.ascontiguousarray(g, dtype=np.float32)

    if abs(float(g[0])) < 1e-30 and np.all(g == g[0]):
        return np.zeros_like(x)

    general = not bool(np.all(g == g[0]))
    xt = x.reshape(TOK, H)
    wupT = np.ascontiguousarray(w_up.T)    # [H, I]
    wdnT = np.ascontiguousarray(w_down.T)  # [I, H]
    if general:
        key = "gen"
        if key not in _NC_CACHE:
            _NC_CACHE[key] = build_nc(True)
        nc = _NC_CACHE[key]
        in_maps = [
            {"x": xt[c * TPC:(c + 1) * TPC], "wupT": wupT, "wdnT": wdnT,
             "g": g}
            for c in range(N_CORES)
        ]
    else:
        g0 = float(g[0])
        key = ("fast", g0)
        if key not in _NC_CACHE:
            _NC_CACHE[key] = build_fast(g0)
        nc = _NC_CACHE[key]
        in_maps = [
            {"x": xt[c * TPC:(c + 1) * TPC], "wupT": wupT, "wdnT": wdnT}
            for c in range(N_CORES)
        ]
    res = run_bass_kernel_spmd(
        nc, in_maps, list(range(N_CORES)),
        trace=bool(os.environ.get("BASS_TRACE")),
    )
    LAST_RESULT = res
    out = np.concatenate([res.results[c]["out"] for c in range(N_CORES)],
                         axis=0)
    return out.reshape(B, S, H)



# revision 13
# speedup vs baseline: 1.2215x; 1.2215x over previous
"""BitNet MLP (act_quant -> ternary matmul -> relu^2 -> SubLN -> act_quant ->
ternary matmul) on 8 Trainium2 NeuronCores, data-parallel over tokens.

Math notes (exactness):
- act_quant int levels (|q| <= 127) and ternary weights {-1,0,1} are exactly
  representable in bf16, so both matmuls run on the PE in bf16 with exact
  integer arithmetic (f32 PSUM accumulation, |sums| < 2^24).
- All quantization scales are folded into per-token scalars applied to the
  final [tok, 512] output: out = i2 * beta_t with
    beta_t = clip(c_t * alpha_t * Sabs_t, 1e-5) * clip(mean|w_dn|,1e-5) / 127
  where alpha_t = (clip(max|x_t|,1e-5) * clip(mean|w_up|,1e-5) / 127)^2,
  Sabs_t = max_i |relu(ih)^2 * g|, c_t = rsqrt(var_t + 1e-6).
- Rounding uses the magic-number trick (x + 1.5*2^23 - 1.5*2^23) == RNE
  round-to-integer for |x| < 2^22, matching jnp.round (half-to-even).
- SubLN variance is recovered from the quantized intermediate:
  var = alpha^2 * sum(iu^2) * (Sabs/127)^2 / (2048 * g0^2); the
  quantization error on sum(iu^2) is ~0.1% which is far below tolerance.
  (For non-constant g an extra pass computes sum((relu^2)^2) directly.)
"""
import os
import numpy as np

import concourse.bass as bass
import concourse.tile as tile
from concourse import mybir
from concourse.bass_utils import run_bass_kernel_spmd
from concourse.masks import make_identity

# ---------------------------------------------------------------------------
# Workaround for walrus "Too many sync wait commands" on the TileContext tail
# drain: split the drain's semaphore waits across single-wait SP NOPs, then
# advance the observed clocks so the real drain needs none.
import re as _re
import bass_rust as _bass_rust


def _patched_drain_and_barrier(self, tick_clock, wait_clock):
    gc = tick_clock.global_clock
    ticks = list(map(int, _re.findall(r"\d+", repr(gc))))
    n = len(ticks)
    nonzero = [(i, t) for i, t in enumerate(ticks) if t > 0]
    for i, t in nonzero:
        sub = [0] * n
        sub[i] = t
        sub_scoped = _bass_rust.ScopedClock({None: _bass_rust.VectorClock(sub)})
        nop = self.nc.sync.nop()
        wait_clock.add_sem_waits(nop.ins, sub_scoped)
        for ec in wait_clock.engine_clocks:
            ec.update_past(sub_scoped)
    drain_inst = self.nc.sync.drain()
    wait_clock.add_sem_waits(drain_inst.ins,
                             _bass_rust.ScopedClock({None: gc}))
    self.nc.all_engine_barrier()
    popped = self.nc._tile_sem_poison_stack.pop()
    assert popped is self._sem_poison
    self.nc.clear_and_free_semaphores(list(self.sems.allocated().values()))
    self.nc.all_engine_barrier()


tile.TileContext._drain_and_barrier = _patched_drain_and_barrier


def _split_sync_waits(nc, keep_default=1):
    """walrus caps the number of semaphore waits a single instruction can
    carry (CTRL ops take only 1; compute ops a few). Hoist excess waits onto
    single-wait NOPs inserted immediately before the instruction on the same
    engine — identical semantics, engines execute in order."""
    import dataclasses
    keep_by_op = {}
    proto = None
    for f in nc.m.functions:
        for bb in f.blocks:
            for inst in bb.instructions:
                if type(inst).__name__ == "InstNoOp":
                    proto = inst
                    break
            if proto is not None:
                break
        if proto is not None:
            break
    counter = [0]
    for f in nc.m.functions:
        new_blocks = []
        for bb in f.blocks:
            out = []
            changed = False
            for inst in bb.instructions:
                si = inst.sync_info
                ow = list(si.on_wait) if si is not None and si.on_wait else []
                keep = keep_by_op.get(inst.opcode, keep_default)
                if len(ow) > keep:
                    assert proto is not None, "no NoOp prototype found yet"
                    for w in ow[:-keep]:
                        counter[0] += 1
                        nop = dataclasses.replace(
                            proto,
                            name=f"I-waitsplit-{counter[0]}",
                            engine=inst.engine,
                            sync_info=_bass_rust.SyncInfo(on_wait=[w],
                                                          on_update=[]),
                        )
                        out.append(nop)
                    si.on_wait = ow[-keep:]
                    changed = True
                out.append(inst)
            if changed:
                bb2 = _bass_rust.BasicBlock(name=bb.name, instructions=out)
                bb2.IsExit = bb.IsExit
                bb2.IsLoopEntry = bb.IsLoopEntry
                bb2.IsPredicated = bb.IsPredicated
                new_blocks.append(bb2)
            else:
                new_blocks.append(bb)
        f.blocks = new_blocks
# ---------------------------------------------------------------------------

F32 = mybir.dt.float32
BF16 = mybir.dt.bfloat16
ALU = mybir.AluOpType
AF = mybir.ActivationFunctionType

N_CORES = 8
B, S, H, I = 8, 8192, 512, 2048
TOK = B * S                  # 65536 tokens total
TPC = TOK // N_CORES         # 8192 tokens per core
P = 128                      # partition tile
NT = TPC // P                # 64 token tiles per core
NKH = H // P                 # 4 k-tiles over H
NKI = I // P                 # 16 k-tiles over I
NB = I // 512                # 4 psum banks for the up matmul

MAGIC = 12582912.0           # 1.5 * 2^23: RNE round-to-int trick
EPS = 1e-6                   # SubLN eps (from reference)

LAST_RESULT = None           # set by kernel() for test harness introspection


def _emit_weight_quant(nc, stage, junkp, ps, consts, wT_dram, n_ktiles,
                       nsub, name, magicb):
    """Quantize a (host-pre-transposed) weight matrix to ternary bf16 tiles.

    wT_dram: [n_ktiles*128, nsub*512] f32 in DRAM (contraction dim on rows).
    Returns (list of [128, nsub, 512] bf16 sbuf tiles, scale_recip [128,1],
    meanclip [128,1]) where meanclip = clip(mean|w|, 1e-5) broadcast to all
    partitions.
    """
    n_elem = n_ktiles * 128 * nsub * 512

    # pass 1: per-partition abs sums
    asum = consts.tile([P, n_ktiles], F32, tag=f"{name}_asum")
    for k in range(n_ktiles):
        wf = stage.tile([P, nsub * 512], F32, tag="stage")
        nc.gpsimd.dma_start(out=wf, in_=wT_dram[k * P:(k + 1) * P, :])
        junk = junkp.tile([P, nsub * 512], BF16, tag="junk")
        nc.scalar.activation(out=junk, in_=wf, func=AF.Abs,
                             accum_out=asum[:, k:k + 1])
    tot = consts.tile([P, 1], F32, tag=f"{name}_tot")
    nc.vector.tensor_reduce(out=tot, in_=asum, axis=mybir.AxisListType.X,
                            op=ALU.add)
    # broadcast-sum across partitions: ones128.T @ tot
    ones128 = stage.tile([P, P], F32, tag="ones128")
    nc.vector.memset(ones128, 1.0)
    totp = ps.tile([P, 1], F32, tag="totp")
    nc.tensor.matmul(out=totp, lhsT=ones128, rhs=tot, start=True, stop=True)
    gsum = consts.tile([P, 1], F32, tag=f"{name}_gsum")
    nc.scalar.copy(out=gsum, in_=totp)
    # mean -> clip -> reciprocal scale
    meanclip = consts.tile([P, 1], F32, tag=f"{name}_meanclip")
    nc.vector.tensor_scalar(out=meanclip, in0=gsum, scalar1=1.0 / n_elem,
                            scalar2=1e-5, op0=ALU.mult, op1=ALU.max)
    swq = consts.tile([P, 1], F32, tag=f"{name}_swq")
    nc.vector.reciprocal(out=swq, in_=meanclip)

    # pass 2: re-load, round+clip to ternary bf16
    wq_tiles = []
    for k in range(n_ktiles):
        wf = stage.tile([P, nsub * 512], F32, tag="stage")
        nc.gpsimd.dma_start(out=wf, in_=wT_dram[k * P:(k + 1) * P, :])
        rt = stage.tile([P, nsub * 512], F32, tag="stage_rt")
        nc.scalar.activation(out=rt, in_=wf, func=AF.Identity,
                             bias=magicb, scale=swq)
        cl = stage.tile([P, nsub * 512], F32, tag="stage_cl")
        nc.vector.tensor_scalar(out=cl, in0=rt, scalar1=MAGIC, scalar2=1.0,
                                op0=ALU.subtract, op1=ALU.min)
        wq = consts.tile([P, nsub, 512], BF16, tag=f"{name}_wq{k}")
        nc.vector.tensor_scalar(out=wq.rearrange("p a b -> p (a b)"), in0=cl,
                                scalar1=-1.0, scalar2=None, op0=ALU.max)
        wq_tiles.append(wq)
    return wq_tiles, meanclip


def build_nc(general_g: bool):
    nc = bass.Bass()
    x_d = nc.dram_tensor("x", [TPC, H], F32, kind="ExternalInput")
    wupT_d = nc.dram_tensor("wupT", [H, I], F32, kind="ExternalInput")
    wdnT_d = nc.dram_tensor("wdnT", [I, H], F32, kind="ExternalInput")
    g_d = nc.dram_tensor("g", [I], F32, kind="ExternalInput")
    out_d = nc.dram_tensor("out", [TPC, H], F32, kind="ExternalOutput")

    from contextlib import ExitStack
    with ExitStack() as ctx:
        tc = ctx.enter_context(tile.TileContext(nc))

        # ---------------- constants / weight prep ----------------
        consts = ctx.enter_context(tc.tile_pool(name="consts", bufs=1))

        ident = consts.tile([P, P], BF16)
        make_identity(nc, ident)

        magicb = consts.tile([P, 1], F32)
        nc.vector.memset(magicb, MAGIC)

        # g broadcast to all partitions: [128, I] f32
        g_bc = consts.tile([P, I], F32)
        g_ap = g_d[:]
        g_bcast_ap = bass.AP(tensor=g_ap.tensor, offset=g_ap.offset,
                             ap=[[0, P]] + list(g_ap.ap))
        nc.gpsimd.dma_start(out=g_bc, in_=g_bcast_ap)

        g0b = consts.tile([P, 1], F32)
        with tc.tile_pool(name="wstage", bufs=2) as stage, \
                tc.tile_pool(name="wjunk", bufs=2) as junkp, \
                tc.tile_pool(name="wps", bufs=1, space="PSUM") as wps:
            # g0 broadcast [128,1] via K=1 matmul with ones
            ones_row = stage.tile([1, P], F32, tag="ones_row")
            nc.vector.memset(ones_row, 1.0)
            g0_sb = stage.tile([1, 1], F32, tag="g0sb")
            nc.gpsimd.dma_start(out=g0_sb, in_=g_d[0:1])
            g0_ps = wps.tile([P, 1], F32, tag="g0ps")
            nc.tensor.matmul(out=g0_ps, lhsT=ones_row, rhs=g0_sb, start=True,
                             stop=True)
            nc.scalar.copy(out=g0b, in_=g0_ps)

            wup_q, up_meanclip = _emit_weight_quant(
                nc, stage, junkp, wps, consts, wupT_d, NKH, NB, "wup", magicb)
            wdn_q, dn_meanclip = _emit_weight_quant(
                nc, stage, junkp, wps, consts, wdnT_d, NKI, 1, "wdn", magicb)

        # k1b = clip(mean|w_up|,1e-5)/127  (per-token gamma multiplier)
        k1b = consts.tile([P, 1], F32)
        nc.vector.tensor_scalar_mul(out=k1b, in0=up_meanclip, scalar1=1.0 / 127.0)
        # wdk = clip(mean|w_dn|,1e-5)/127  (final output multiplier)
        wdk = consts.tile([P, 1], F32)
        nc.vector.tensor_scalar_mul(out=wdk, in0=dn_meanclip, scalar1=1.0 / 127.0)
        # sg127 = sign(g0)*127 (quant scale sign), g0a = |g0|
        sg127 = consts.tile([P, 1], F32)
        nc.scalar.activation(out=sg127, in_=g0b, func=AF.Sign)
        nc.vector.tensor_scalar_mul(out=sg127, in0=sg127, scalar1=127.0)
        g0a = consts.tile([P, 1], F32)
        nc.scalar.activation(out=g0a, in_=g0b, func=AF.Abs)

        # ---------------- main token-tile pipeline ----------------
        # isg = sign(g0)/127 (or 1/127 for general g): folds the quant scale
        # sign so d = recip(clip(S)*isg) = sign*127/clip(S) in 2 small ops.
        isg = consts.tile([P, 1], F32)
        if general_g:
            nc.vector.memset(isg, 1.0 / 127.0)
        else:
            nc.vector.tensor_scalar_mul(out=isg, in0=sg127,
                                        scalar1=1.0 / (127.0 * 127.0))

        BG = 8  # tiles per small-op batch
        KV = 1.0 / (127.0 * 127.0 * I)

        xs_pool = ctx.enter_context(tc.tile_pool(name="xs", bufs=2 * BG))
        xq_pool = ctx.enter_context(tc.tile_pool(name="xqp", bufs=3))
        big = ctx.enter_context(tc.tile_pool(name="big", bufs=2))
        iup = ctx.enter_context(tc.tile_pool(name="iup", bufs=3))
        outp = ctx.enter_context(tc.tile_pool(name="outp", bufs=BG + 1))
        o2p = ctx.enter_context(tc.tile_pool(name="o2p", bufs=3))
        junkp = ctx.enter_context(tc.tile_pool(name="mjunk", bufs=1))
        small = ctx.enter_context(tc.tile_pool(name="small", bufs=3))
        batchp = ctx.enter_context(tc.tile_pool(name="batchp", bufs=2))
        ps_xT = ctx.enter_context(tc.tile_pool(name="ps_xT", bufs=1,
                                               space="PSUM"))
        ps_ih = ctx.enter_context(tc.tile_pool(name="ps_ih", bufs=1,
                                               space="PSUM"))
        ps_iuT = ctx.enter_context(tc.tile_pool(name="ps_iuT", bufs=1,
                                                space="PSUM"))
        ps_o = ctx.enter_context(tc.tile_pool(name="ps_o", bufs=1,
                                              space="PSUM"))

        IH2 = I // 2  # up-matmul accumulates in two 2-bank halves

        def phase_a(ib, state):
            """DMA x tiles + per-token absmax, then batched x-scale chain."""
            xm8 = batchp.tile([P, BG], F32, tag="xm8")
            x_tiles = []
            for j in range(BG):
                r0 = (ib + j) * P
                x_sb = xs_pool.tile([P, H], F32, tag="x")
                nc.sync.dma_start(out=x_sb, in_=x_d[r0:r0 + P, :])
                x_tiles.append(x_sb)
                nc.vector.tensor_reduce(out=xm8[:, j:j + 1], in_=x_sb,
                                        axis=mybir.AxisListType.X, op=ALU.max,
                                        apply_absolute_value=True)
            t08 = batchp.tile([P, BG], F32, tag="t08")
            nc.vector.tensor_scalar_max(out=t08, in0=xm8, scalar1=1e-5)
            xr8 = batchp.tile([P, BG], F32, tag="xr8")
            nc.vector.reciprocal(out=xr8, in_=t08)
            xsc8 = batchp.tile([P, BG], F32, tag="xsc8")
            nc.vector.tensor_scalar_mul(out=xsc8, in0=xr8, scalar1=127.0)
            state[ib] = (x_tiles, t08, xsc8)

        def phase_bc(ib, state):
            x_tiles, t08, xsc8 = state.pop(ib)
            Sm8 = batchp.tile([P, BG], F32, tag="Sm8")
            q28 = batchp.tile([P, BG], F32, tag="q28")
            q2g8 = None
            if general_g:
                q2g8 = batchp.tile([P, BG], F32, tag="q2g8")
            o_tiles = []

            for j in range(BG):
                x_sb = x_tiles[j]
                # quantize x (RNE round via magic): ACT + DVE
                xq = xq_pool.tile([P, H], F32, tag="xq")
                nc.scalar.activation(out=xq, in_=x_sb, func=AF.Identity,
                                     bias=magicb, scale=xsc8[:, j:j + 1])
                ix = xq_pool.tile([P, H], BF16, tag="ix")
                nc.vector.tensor_scalar(out=ix, in0=xq, scalar1=MAGIC,
                                        scalar2=None, op0=ALU.subtract)
                # transpose ix via PE, drain on ACT
                xT_ps = ps_xT.tile([P, NKH, P], BF16, tag="xT")
                for k in range(NKH):
                    nc.tensor.transpose(out=xT_ps[:, k, :],
                                        in_=ix[:, k * P:(k + 1) * P],
                                        identity=ident)
                xT_sb = xq_pool.tile([P, NKH, P], BF16, tag="xTsb")
                nc.scalar.copy(out=xT_sb, in_=xT_ps)

                # up matmul in two halves (each 2 PSUM banks) so the next
                # tile's matmuls only wait on a half-drain
                r_sb = big.tile([P, I], F32, tag="r")
                smh = small.tile([P, 2], F32, tag="smh")
                for h in range(2):
                    ihh = ps_ih.tile([P, IH2], F32, tag="ih")
                    for nb in range(2):
                        lo = nb * 512
                        for k in range(NKH):
                            nc.tensor.matmul(
                                out=ihh[:, lo:lo + 512],
                                lhsT=xT_sb[:, k, :],
                                rhs=wup_q[k][:, 2 * h + nb, :],
                                start=(k == 0), stop=(k == NKH - 1))
                    nc.scalar.activation(out=r_sb[:, h * IH2:(h + 1) * IH2],
                                         in_=ihh, func=AF.Relu)
                    if not general_g:
                        nc.vector.tensor_reduce(out=smh[:, h:h + 1], in_=ihh,
                                                axis=mybir.AxisListType.X,
                                                op=ALU.max)

                if general_g:
                    s_sb = big.tile([P, I], F32, tag="s")
                    nc.gpsimd.tensor_tensor(out=s_sb, in0=r_sb, in1=r_sb,
                                            op=ALU.mult)
                    sq_in = big.tile([P, I], F32, tag="sg")
                    nc.vector.tensor_tensor(out=sq_in, in0=s_sb, in1=g_bc,
                                            op=ALU.mult)
                    junk3 = junkp.tile([P, I], BF16, tag="junk3")
                    nc.scalar.activation(out=junk3, in_=s_sb, func=AF.Square,
                                         accum_out=q2g8[:, j:j + 1])
                    nc.vector.tensor_reduce(out=Sm8[:, j:j + 1], in_=sq_in,
                                            axis=mybir.AxisListType.X,
                                            op=ALU.max,
                                            apply_absolute_value=True)
                    sc2 = small.tile([P, 1], F32, tag="sc2")
                    nc.vector.tensor_scalar(out=sc2, in0=Sm8[:, j:j + 1],
                                            scalar1=1e-30, scalar2=isg,
                                            op0=ALU.max, op1=ALU.mult)
                    dr = small.tile([P, 1], F32, tag="dr")
                    nc.vector.reciprocal(out=dr, in_=sc2)
                    rt = big.tile([P, I], F32, tag="rt")
                    nc.vector.tensor_scalar(out=rt, in0=sq_in, scalar1=dr,
                                            scalar2=MAGIC, op0=ALU.mult,
                                            op1=ALU.add)
                    iu = iup.tile([P, I], BF16, tag="iu")
                    nc.vector.tensor_scalar(out=iu, in0=rt, scalar1=MAGIC,
                                            scalar2=None, op0=ALU.subtract)
                else:
                    # s' = r*r on GPSIMD
                    s_sb = big.tile([P, I], F32, tag="s")
                    nc.gpsimd.tensor_tensor(out=s_sb, in0=r_sb, in1=r_sb,
                                            op=ALU.mult)
                    # rmax = max over both halves (straight from PSUM above)
                    nc.vector.tensor_reduce(out=Sm8[:, j:j + 1], in_=smh,
                                            axis=mybir.AxisListType.X,
                                            op=ALU.max)
                    mr = small.tile([P, 1], F32, tag="mr")
                    nc.vector.tensor_scalar_max(out=mr, in0=Sm8[:, j:j + 1],
                                                scalar1=1e-15)
                    sc2 = small.tile([P, 1], F32, tag="sc2")
                    nc.vector.tensor_scalar(out=sc2, in0=mr, scalar1=mr,
                                            scalar2=isg, op0=ALU.mult,
                                            op1=ALU.mult)
                    dr = small.tile([P, 1], F32, tag="dr")
                    nc.vector.reciprocal(out=dr, in_=sc2)
                    rt = big.tile([P, I], F32, tag="rt")
                    nc.vector.tensor_scalar(out=rt, in0=s_sb, scalar1=dr,
                                            scalar2=MAGIC, op0=ALU.mult,
                                            op1=ALU.add)
                    iu = iup.tile([P, I], BF16, tag="iu")
                    nc.vector.tensor_scalar(out=iu, in0=rt, scalar1=MAGIC,
                                            scalar2=None, op0=ALU.subtract)

                # q2 = sum(iu^2) (ACT square + accumulate)
                junk2 = junkp.tile([P, I], BF16, tag="junk2")
                nc.scalar.activation(out=junk2, in_=iu, func=AF.Square,
                                     accum_out=q28[:, j:j + 1])

                # transpose iu via PE, drain on ACT
                iuT_ps = ps_iuT.tile([P, NKI, P], BF16, tag="iuT")
                for k in range(NKI):
                    nc.tensor.transpose(out=iuT_ps[:, k, :],
                                        in_=iu[:, k * P:(k + 1) * P],
                                        identity=ident)
                iuT_sb = iup.tile([P, NKI, P], BF16, tag="iuTsb")
                nc.scalar.copy(out=iuT_sb, in_=iuT_ps)

                # down matmul + plain drain (beta applied later, batched)
                o_ps = ps_o.tile([P, H], F32, tag="o")
                for k in range(NKI):
                    nc.tensor.matmul(out=o_ps, lhsT=iuT_sb[:, k, :],
                                     rhs=wdn_q[k][:, 0, :],
                                     start=(k == 0), stop=(k == NKI - 1))
                o_sb = outp.tile([P, H], F32, tag="osb")
                nc.scalar.copy(out=o_sb, in_=o_ps)
                o_tiles.append(o_sb)

            # --- batched beta chain ---
            scc8 = batchp.tile([P, BG], F32, tag="scc8")
            if general_g:
                nc.vector.tensor_scalar_max(out=scc8, in0=Sm8, scalar1=1e-30)
            else:
                ra8 = batchp.tile([P, BG], F32, tag="ra8")
                nc.vector.tensor_scalar_max(out=ra8, in0=Sm8, scalar1=0.0)
                ssq8 = batchp.tile([P, BG], F32, tag="ssq8")
                nc.vector.tensor_tensor(out=ssq8, in0=ra8, in1=ra8,
                                        op=ALU.mult)
                nc.vector.tensor_scalar_max(out=scc8, in0=ssq8,
                                            scalar1=1e-30)
            ga8 = batchp.tile([P, BG], F32, tag="ga8")
            nc.vector.tensor_scalar_mul(out=ga8, in0=t08, scalar1=k1b)
            al8 = batchp.tile([P, BG], F32, tag="al8")
            nc.vector.tensor_tensor(out=al8, in0=ga8, in1=ga8, op=ALU.mult)
            m18 = batchp.tile([P, BG], F32, tag="m18")
            nc.vector.tensor_tensor(out=m18, in0=al8, in1=scc8, op=ALU.mult)
            v18 = batchp.tile([P, BG], F32, tag="v18")
            Ve8 = batchp.tile([P, BG], F32, tag="Ve8")
            if general_g:
                al28 = batchp.tile([P, BG], F32, tag="al28")
                nc.vector.tensor_tensor(out=al28, in0=al8, in1=al8,
                                        op=ALU.mult)
                nc.vector.tensor_tensor(out=v18, in0=al28, in1=q2g8,
                                        op=ALU.mult)
                nc.vector.tensor_scalar(out=Ve8, in0=v18, scalar1=1.0 / I,
                                        scalar2=EPS, op0=ALU.mult,
                                        op1=ALU.add)
            else:
                m28 = batchp.tile([P, BG], F32, tag="m28")
                nc.vector.tensor_tensor(out=m28, in0=m18, in1=m18,
                                        op=ALU.mult)
                nc.vector.tensor_tensor(out=v18, in0=m28, in1=q28,
                                        op=ALU.mult)
                nc.vector.tensor_scalar(out=Ve8, in0=v18, scalar1=KV,
                                        scalar2=EPS, op0=ALU.mult,
                                        op1=ALU.add)
            sq8 = batchp.tile([P, BG], F32, tag="sq8")
            nc.scalar.activation(out=sq8, in_=Ve8, func=AF.Sqrt)
            cr8 = batchp.tile([P, BG], F32, tag="cr8")
            nc.vector.reciprocal(out=cr8, in_=sq8)
            h18 = batchp.tile([P, BG], F32, tag="h18")
            nc.vector.tensor_tensor(out=h18, in0=cr8, in1=cr8, op=ALU.mult)
            h28 = batchp.tile([P, BG], F32, tag="h28")
            nc.vector.tensor_tensor(out=h28, in0=h18, in1=Ve8, op=ALU.mult)
            h38 = batchp.tile([P, BG], F32, tag="h38")
            nc.vector.tensor_scalar(out=h38, in0=h28, scalar1=-0.5,
                                    scalar2=1.5, op0=ALU.mult, op1=ALU.add)
            c8 = batchp.tile([P, BG], F32, tag="c8")
            nc.vector.tensor_tensor(out=c8, in0=cr8, in1=h38, op=ALU.mult)
            if general_g:
                m1g8 = m18
            else:
                m1g8 = batchp.tile([P, BG], F32, tag="m1g8")
                nc.vector.tensor_scalar_mul(out=m1g8, in0=m18, scalar1=g0a)
            mu8 = batchp.tile([P, BG], F32, tag="mu8")
            nc.vector.tensor_tensor(out=mu8, in0=c8, in1=m1g8, op=ALU.mult)
            b8 = batchp.tile([P, BG], F32, tag="b8")
            nc.vector.tensor_scalar(out=b8, in0=mu8, scalar1=1e-5,
                                    scalar2=wdk, op0=ALU.max, op1=ALU.mult)

            # --- scale + store ---
            for j in range(BG):
                r0 = (ib + j) * P
                o2 = o2p.tile([P, H], F32, tag="o2")
                nc.vector.tensor_scalar_mul(out=o2, in0=o_tiles[j],
                                            scalar1=b8[:, j:j + 1])
                nc.sync.dma_start(out=out_d[r0:r0 + P, :], in_=o2)

        # software-pipelined emission: batch ib+1's loads are issued before
        # batch ib's compute so DMA/absmax overlap the previous batch
        state = {}
        phase_a(0, state)
        for ib in range(0, NT, BG):
            if ib + BG < NT:
                phase_a(ib + BG, state)
            phase_bc(ib, state)

    _split_sync_waits(nc)
    return nc


def build_fast(g0: float):
    """Const-g fast path.

    Per 128-token tile (exact integer math, scales folded into final beta):
      DVE : xq = x*sc + MAGIC           (round-to-int via magic, f32)
      ACT : ix = xq - MAGIC -> bf16     (exact int8 levels)
      PE  : xT = transpose(ix)          (4x N=128)
      PE  : h  = ixT.T @ wup_q          (16x N=512 bf16, exact ints in PSUM)
      DVE : s2 = max(h,0)*h             (= relu(h)^2, one STT from PSUM)
      DVE : S2m = max(s2)               (= Rm^2, feeds d = 127/Rm^2)
      ACT : t1 = s2*d + MAGIC           (per-token scale via ACT scale port)
      DVE : iu = t1 - MAGIC -> bf16     (exact int levels 0..127)
      GPS : q2 = sum(iu^2)              (STT with accum, junk main output)
      PE  : iuT = transpose(iu)         (16x N=128), ACT drains
      PE  : o  = iuT.T @ wdn_q          (16x N=512)
      ACT : o_sb = copy(o)
      DVE : out = o_sb * beta, DMA out  (beta via batched per-8 chain)
    Down-matmuls are emitted one 4-tile group behind the up-matmuls so the
    PE never waits on the s2->iu chain.
    """
    nc = bass.Bass()
    x_d = nc.dram_tensor("x", [TPC, H], F32, kind="ExternalInput")
    wupT_d = nc.dram_tensor("wupT", [H, I], F32, kind="ExternalInput")
    wdnT_d = nc.dram_tensor("wdnT", [I, H], F32, kind="ExternalInput")
    out_d = nc.dram_tensor("out", [TPC, H], F32, kind="ExternalOutput")

    BG = 8          # stats/beta batch
    GRP = 4         # pipeline group (down-matmul lag)
    IH2 = I // 2
    KV = 1.0 / (127.0 * 127.0 * I)
    g0a = abs(g0)
    g0s = 1.0 if g0 >= 0 else -1.0

    from contextlib import ExitStack
    with ExitStack() as ctx:
        tc = ctx.enter_context(tile.TileContext(nc))

        consts = ctx.enter_context(tc.tile_pool(name="consts", bufs=1))
        ident = consts.tile([P, P], BF16)
        make_identity(nc, ident)
        identf = consts.tile([P, P], F32)
        make_identity(nc, identf)
        wup_q = consts.tile([P, NKH, I], BF16)
        wdn_q = consts.tile([P, NKI, H], BF16)
        k1b = consts.tile([P, 1], F32)
        wdk = consts.tile([P, 1], F32)
        magicb = consts.tile([P, 1], F32)
        nc.vector.memset(magicb, MAGIC)
        nmagicb = consts.tile([P, 1], F32)
        nc.vector.memset(nmagicb, -MAGIC)

        # pools that must exist before weight prep so x loads / absmax /
        # quant / transposes overlap the prologue
        xs_pool = ctx.enter_context(tc.tile_pool(name="xs", bufs=9))
        batchp = ctx.enter_context(tc.tile_pool(name="batchp", bufs=3))
        xq_pool = ctx.enter_context(tc.tile_pool(name="xqp", bufs=3))
        ix_pool = ctx.enter_context(tc.tile_pool(name="ixp", bufs=3))
        xT_pool = ctx.enter_context(tc.tile_pool(name="xTp", bufs=12))
        ps_tp = ctx.enter_context(tc.tile_pool(name="ps_tp", bufs=2,
                                               space="PSUM"))

        state = {}

        def load_batch(ib):
            xm8 = batchp.tile([P, BG], F32, tag="xm8")
            x_tiles = []
            for jj in range(BG):
                r0 = (ib + jj) * P
                x_sb = xs_pool.tile([P, H], F32, tag="x")
                nc.sync.dma_start(out=x_sb, in_=x_d[r0:r0 + P, :])
                nc.vector.tensor_reduce(out=xm8[:, jj:jj + 1], in_=x_sb,
                                        axis=mybir.AxisListType.X, op=ALU.max,
                                        apply_absolute_value=True)
                x_tiles.append(x_sb)
            t08 = batchp.tile([P, BG], F32, tag="t08")
            nc.vector.tensor_scalar_max(out=t08, in0=xm8, scalar1=1e-5)
            xr8 = batchp.tile([P, BG], F32, tag="xr8")
            nc.vector.reciprocal(out=xr8, in_=t08)
            xsc8 = batchp.tile([P, BG], F32, tag="xsc8")
            nc.vector.tensor_scalar_mul(out=xsc8, in0=xr8, scalar1=127.0)
            S2m8 = batchp.tile([P, BG], F32, tag="S2m8")
            q28 = batchp.tile([P, BG], F32, tag="q28")
            state[ib] = dict(x=x_tiles, t08=t08, xsc8=xsc8, S2m8=S2m8,
                             q28=q28)

        load_batch(0)
        load_batch(BG)
        tstate = {}

        def front_x(j):
            ib = (j // BG) * BG
            jj = j - ib
            st = state[ib]
            x_sb = st["x"][jj]
            xq = xq_pool.tile([P, H], F32, tag="xq")
            nc.vector.tensor_scalar(out=xq, in0=x_sb,
                                    scalar1=st["xsc8"][:, jj:jj + 1],
                                    scalar2=MAGIC, op0=ALU.mult, op1=ALU.add)
            ixt = ix_pool.tile([P, H], BF16, tag="ix")
            nc.scalar.activation(out=ixt, in_=xq, func=AF.Identity,
                                 bias=nmagicb)
            tp = ps_tp.tile([P, 8, P], BF16, tag="tp")
            for k in range(NKH):
                nc.tensor.transpose(out=tp[:, k, :],
                                    in_=ixt[:, k * P:(k + 1) * P],
                                    identity=ident)
            xT = xT_pool.tile([P, NKH, P], BF16, tag="xT")
            nc.scalar.copy(out=xT, in_=tp[:, :NKH, :])
            tstate[j] = dict(xT=xT)

        for _j in range(2 * GRP):
            front_x(_j)

        # ---------------- weight prep (single load) ----------------
        with tc.tile_pool(name="wstage", bufs=1) as wst, \
                tc.tile_pool(name="wscr", bufs=2) as wsc, \
                tc.tile_pool(name="wjunk", bufs=2) as wjk, \
                tc.tile_pool(name="wps", bufs=1, space="PSUM") as wps:
            up_st = wst.tile([P, NKH, I], F32, tag="upst")
            dn_st = wst.tile([P, NKI, H], F32, tag="dnst")
            for k in range(NKH):
                nc.gpsimd.dma_start(out=up_st[:, k, :],
                                    in_=wupT_d[k * P:(k + 1) * P, :])

            asum_u = consts.tile([P, NKH], F32)
            asum_d = consts.tile([P, NKH], F32)
            for k in range(NKH):
                jku = wjk.tile([P, I], BF16, tag="jk")
                nc.scalar.activation(out=jku, in_=up_st[:, k, :], func=AF.Abs,
                                     accum_out=asum_u[:, k:k + 1])
            ones128 = wsc.tile([P, P], F32, tag="ones", bufs=1)
            nc.vector.memset(ones128, 1.0)

            def total_meanclip(asum, n_elem, mc_out):
                tot = consts.tile([P, 1], F32)
                nc.vector.tensor_reduce(out=tot, in_=asum,
                                        axis=mybir.AxisListType.X, op=ALU.add)
                totp = wps.tile([P, 1], F32, tag="totp")
                nc.tensor.matmul(out=totp, lhsT=ones128, rhs=tot, start=True,
                                 stop=True)
                gsum = consts.tile([P, 1], F32)
                nc.scalar.copy(out=gsum, in_=totp)
                nc.vector.tensor_scalar(out=mc_out, in0=gsum,
                                        scalar1=1.0 / n_elem, scalar2=1e-5,
                                        op0=ALU.mult, op1=ALU.max)

            mc_u = consts.tile([P, 1], F32)
            mc_d = consts.tile([P, 1], F32)
            total_meanclip(asum_u, H * I, mc_u)
            swq_u = consts.tile([P, 1], F32)
            nc.vector.reciprocal(out=swq_u, in_=mc_u)
            nc.vector.tensor_scalar_mul(out=k1b, in0=mc_u, scalar1=1.0 / 127.0)

            def quant_chunk(src, dst, swq):
                qt = wsc.tile(list(src.shape), F32, tag="qt")
                nc.scalar.activation(out=qt, in_=src, func=AF.Identity,
                                     bias=magicb, scale=swq)
                qu = wsc.tile(list(src.shape), F32, tag="qu")
                nc.vector.tensor_scalar(out=qu, in0=qt, scalar1=MAGIC,
                                        scalar2=1.0, op0=ALU.subtract,
                                        op1=ALU.min)
                nc.vector.tensor_scalar(out=dst, in0=qu, scalar1=-1.0,
                                        scalar2=None, op0=ALU.max)

            for k in range(NKH):
                quant_chunk(up_st[:, k, :], wup_q[:, k, :], swq_u)

            # --- wdn prep after wup so the first up-matmuls start earlier
            for k in range(NKI):
                nc.gpsimd.dma_start(out=dn_st[:, k, :],
                                    in_=wdnT_d[k * P:(k + 1) * P, :])
            for k in range(NKH):
                jkd = wjk.tile([P, NKH, H], BF16, tag="jkd")
                nc.scalar.activation(out=jkd,
                                     in_=dn_st[:, k * NKH:(k + 1) * NKH, :],
                                     func=AF.Abs,
                                     accum_out=asum_d[:, k:k + 1])
            total_meanclip(asum_d, H * I, mc_d)
            swq_d = consts.tile([P, 1], F32)
            nc.vector.reciprocal(out=swq_d, in_=mc_d)
            nc.vector.tensor_scalar_mul(out=wdk, in0=mc_d, scalar1=1.0 / 127.0)
            for k in range(NKH):
                quant_chunk(dn_st[:, k * NKH:(k + 1) * NKH, :],
                            wdn_q[:, k * NKH:(k + 1) * NKH, :], swq_d)

        # ---------------- main-loop pools ----------------
        r_pool = ctx.enter_context(tc.tile_pool(name="rp", bufs=3))
        s2_pool = ctx.enter_context(tc.tile_pool(name="s2p", bufs=2))
        t1_pool = ctx.enter_context(tc.tile_pool(name="t1p", bufs=2))
        iu_pool = ctx.enter_context(tc.tile_pool(name="iup", bufs=8))
        iuT_pool = ctx.enter_context(tc.tile_pool(name="iuTp", bufs=5))
        dj_pool = ctx.enter_context(tc.tile_pool(name="djp", bufs=2))
        o_pool = ctx.enter_context(tc.tile_pool(name="op", bufs=8))
        o2_pool = ctx.enter_context(tc.tile_pool(name="o2p", bufs=4))
        d_pool = ctx.enter_context(tc.tile_pool(name="dp", bufs=2))
        bb_pool = ctx.enter_context(tc.tile_pool(name="bbp", bufs=2))
        ps_h = ctx.enter_context(tc.tile_pool(name="ps_h", bufs=2,
                                              space="PSUM"))
        ps_o = ctx.enter_context(tc.tile_pool(name="ps_o", bufs=2,
                                              space="PSUM"))

        def front_mm(j):
            ib = (j // BG) * BG
            jj = j - ib
            st = state[ib]
            xT = tstate[j].pop("xT")
            r = r_pool.tile([P, I], F32, tag="r")
            for half in range(2):
                hh = ps_h.tile([P, IH2], F32, tag="h")
                for nb in range(2):
                    lo = nb * 512
                    for k in range(NKH):
                        nc.tensor.matmul(
                            out=hh[:, lo:lo + 512],
                            lhsT=xT[:, k, :],
                            rhs=wup_q[:, k, (2 * half + nb) * 512:
                                      (2 * half + nb + 1) * 512],
                            start=(k == 0), stop=(k == NKH - 1))
                nc.scalar.activation(out=r[:, half * IH2:(half + 1) * IH2],
                                     in_=hh, func=AF.Relu)
            # per-token Rm = max(relu(h)) (>= 0 since r >= 0)
                nc.vector.tensor_reduce(out=st["S2m8"][:, jj:jj + 1], in_=r,
                                    axis=mybir.AxisListType.X, op=ALU.max)
            # s2 = relu(h)^2 off the critical DVE/ACT paths
            s2 = s2_pool.tile([P, I], F32, tag="s2")
            nc.gpsimd.tensor_tensor(out=s2, in0=r, in1=r, op=ALU.mult)
            tstate[j]["s2"] = s2

        def dbatch(b):
            ib = (b // BG) * BG
            jj0 = b - ib
            S2m8 = state[ib]["S2m8"]
            mr4 = d_pool.tile([P, GRP], F32, tag="mr4")
            nc.vector.tensor_scalar_max(out=mr4, in0=S2m8[:, jj0:jj0 + GRP],
                                        scalar1=1e-15)
            sc4 = d_pool.tile([P, GRP], F32, tag="sc4")
            nc.vector.tensor_tensor(out=sc4, in0=mr4, in1=mr4, op=ALU.mult)
            sc4b = d_pool.tile([P, GRP], F32, tag="sc4b")
            nc.vector.tensor_scalar_mul(out=sc4b, in0=sc4,
                                        scalar1=1.0 / 127.0)
            d4 = d_pool.tile([P, GRP], F32, tag="d4")
            nc.vector.reciprocal(out=d4, in_=sc4b)
            for j in range(b, b + GRP):
                tstate[j]["d"] = d4[:, j - b:j - b + 1]

        def quant_a(j):
            ts = tstate[j]
            t1 = t1_pool.tile([P, I], F32, tag="t1")
            nc.scalar.activation(out=t1, in_=ts["s2"], func=AF.Identity,
                                 bias=magicb, scale=ts["d"])
            iu = iu_pool.tile([P, I], BF16, tag="iu")
            nc.vector.tensor_scalar(out=iu, in0=t1, scalar1=MAGIC,
                                    scalar2=None, op0=ALU.subtract)
            ts["iu"] = iu
            del ts["s2"]

        def quant_b(j):
            ts = tstate[j]
            iu = ts.pop("iu")
            iuT = iuT_pool.tile([P, NKI, P], BF16, tag="iuT")
            for half in range(2):
                tp2 = ps_tp.tile([P, 8, P], BF16, tag="tp")
                for c in range(8):
                    kk = half * 8 + c
                    nc.tensor.transpose(out=tp2[:, c, :],
                                        in_=iu[:, kk * P:(kk + 1) * P],
                                        identity=ident)
                nc.scalar.copy(out=iuT[:, half * 8:(half + 1) * 8, :],
                               in_=tp2)
            ts["iuT"] = iuT

        def down(j):
            ib = (j // BG) * BG
            jj = j - ib
            ts = tstate[j]
            o_ps = ps_o.tile([P, H], F32, tag="o")
            for k in range(NKI):
                nc.tensor.matmul(out=o_ps, lhsT=ts["iuT"][:, k, :],
                                 rhs=wdn_q[:, k, :],
                                 start=(k == 0), stop=(k == NKI - 1))
            o_sb = o_pool.tile([P, H], F32, tag="o_sb")
            nc.scalar.copy(out=o_sb, in_=o_ps)
            # q2 = sum(iu^2) via the diagonal of iuT.T @ iuT on the PE
            dg_ps = ps_tp.tile([P, P], F32, tag="tp")
            for k in range(NKI):
                nc.tensor.matmul(out=dg_ps, lhsT=ts["iuT"][:, k, :],
                                 rhs=ts["iuT"][:, k, :],
                                 start=(k == 0), stop=(k == NKI - 1))
            dj = dj_pool.tile([P, P], F32, tag="dj")
            nc.vector.tensor_tensor(out=dj, in0=dg_ps, in1=identf,
                                    op=ALU.mult)
            nc.vector.tensor_reduce(out=state[ib]["q28"][:, jj:jj + 1],
                                    in_=dj, axis=mybir.AxisListType.X,
                                    op=ALU.add)
            ts["o"] = o_sb
            del ts["iuT"]

        def bbatch(ib):
            st = state[ib]
            ga8 = bb_pool.tile([P, BG], F32, tag="ga8")
            nc.vector.tensor_scalar_mul(out=ga8, in0=st["t08"], scalar1=k1b)
            al8 = bb_pool.tile([P, BG], F32, tag="al8")
            nc.vector.tensor_tensor(out=al8, in0=ga8, in1=ga8, op=ALU.mult)
            ssq8 = bb_pool.tile([P, BG], F32, tag="ssq8")
            nc.vector.tensor_tensor(out=ssq8, in0=st["S2m8"], in1=st["S2m8"],
                                    op=ALU.mult)
            scc8 = bb_pool.tile([P, BG], F32, tag="scc8")
            nc.vector.tensor_scalar_max(out=scc8, in0=ssq8, scalar1=1e-30)
            m18 = bb_pool.tile([P, BG], F32, tag="m18")
            nc.vector.tensor_tensor(out=m18, in0=al8, in1=scc8, op=ALU.mult)
            m28 = bb_pool.tile([P, BG], F32, tag="m28")
            nc.vector.tensor_tensor(out=m28, in0=m18, in1=m18, op=ALU.mult)
            v18 = bb_pool.tile([P, BG], F32, tag="v18")
            nc.vector.tensor_tensor(out=v18, in0=m28, in1=st["q28"],
                                    op=ALU.mult)
            Ve8 = bb_pool.tile([P, BG], F32, tag="Ve8")
            nc.vector.tensor_scalar(out=Ve8, in0=v18, scalar1=KV,
                                    scalar2=EPS, op0=ALU.mult, op1=ALU.add)
            sq8 = bb_pool.tile([P, BG], F32, tag="sq8")
            nc.scalar.activation(out=sq8, in_=Ve8, func=AF.Sqrt)
            cr8 = bb_pool.tile([P, BG], F32, tag="cr8")
            nc.vector.reciprocal(out=cr8, in_=sq8)
            h18 = bb_pool.tile([P, BG], F32, tag="h18")
            nc.vector.tensor_tensor(out=h18, in0=cr8, in1=cr8, op=ALU.mult)
            h28 = bb_pool.tile([P, BG], F32, tag="h28")
            nc.vector.tensor_tensor(out=h28, in0=h18, in1=Ve8, op=ALU.mult)
            h38 = bb_pool.tile([P, BG], F32, tag="h38")
            nc.vector.tensor_scalar(out=h38, in0=h28, scalar1=-0.5,
                                    scalar2=1.5, op0=ALU.mult, op1=ALU.add)
            c8 = bb_pool.tile([P, BG], F32, tag="c8")
            nc.vector.tensor_tensor(out=c8, in0=cr8, in1=h38, op=ALU.mult)
            mu8 = bb_pool.tile([P, BG], F32, tag="mu8")
            nc.vector.tensor_tensor(out=mu8, in0=c8, in1=m18, op=ALU.mult)
            mc8 = bb_pool.tile([P, BG], F32, tag="mc8")
            nc.vector.tensor_scalar(out=mc8, in0=mu8, scalar1=g0a,
                                    scalar2=1e-5, op0=ALU.mult, op1=ALU.max)
            b8 = bb_pool.tile([P, BG], F32, tag="b8")
            nc.vector.tensor_scalar(out=b8, in0=mc8, scalar1=wdk,
                                    scalar2=g0s, op0=ALU.mult, op1=ALU.mult)
            st["b8"] = b8

        def outt(j):
            ib = (j // BG) * BG
            jj = j - ib
            ts = tstate.pop(j)
            b8 = state[ib]["b8"]
            o2 = o2_pool.tile([P, H], F32, tag="o2")
            nc.vector.tensor_scalar_mul(out=o2, in0=ts["o"],
                                        scalar1=b8[:, jj:jj + 1])
            nc.sync.dma_start(out=out_d[j * P:(j + 1) * P, :], in_=o2)

        for b in range(0, NT, GRP):
            if b % BG == 0 and b >= BG and b + BG < NT:
                load_batch(b + BG)
            if b + 2 * GRP <= NT - GRP + 3:
                for j in range(b + 2 * GRP, min(b + 3 * GRP, NT)):
                    front_x(j)
            for j in range(b, b + GRP):
                front_mm(j)
            dbatch(b)
            if b >= GRP:
                for j in range(b - GRP, b):
                    quant_b(j)
                for j in range(b - GRP, b):
                    down(j)
            if b % BG == 0 and b >= 2 * GRP:
                bbatch(b - 2 * GRP)
            if b >= 2 * GRP:
                for j in range(b - 2 * GRP, b - GRP):
                    outt(j)
            for j in range(b, b + GRP):
                quant_a(j)
        for j in range(NT - GRP, NT):
            quant_b(j)
        for j in range(NT - GRP, NT):
            down(j)
        bbatch(NT - 2 * GRP)
        for j in range(NT - 2 * GRP, NT):
            outt(j)

    _split_sync_waits(nc)
    return nc


_NC_CACHE = {}


def kernel(x, w_up, w_down, g):
    global LAST_RESULT
    x = np.ascontiguousarray(x, dtype=np.float32)
    w_up = np.ascontiguousarray(w_up, dtype=np.float32)
    w_down = np.ascontiguousarray(w_down, dtype=np.float32)
    g = np.ascontiguousarray(g, dtype=np.float32)

    if abs(float(g[0])) < 1e-30 and np.all(g == g[0]):
        return np.zeros_like(x)

    general = not bool(np.all(g == g[0]))
    xt = x.reshape(TOK, H)
    wupT = np.ascontiguousarray(w_up.T)    # [H, I]
    wdnT = np.ascontiguousarray(w_down.T)  # [I, H]
    if general:
        key = "gen"
        if key not in _NC_CACHE:
            _NC_CACHE[key] = build_nc(True)
        nc = _NC_CACHE[key]
        in_maps = [
            {"x": xt[c * TPC:(c + 1) * TPC], "wupT": wupT, "wdnT": wdnT,
             "g": g}
            for c in range(N_CORES)
        ]
    else:
        g0 = float(g[0])
        key = ("fast", g0)
        if key not in _NC_CACHE:
            _NC_CACHE[key] = build_fast(g0)
        nc = _NC_CACHE[key]
        in_maps = [
            {"x": xt[c * TPC:(c + 1) * TPC], "wupT": wupT, "wdnT": wdnT}
            for c in range(N_CORES)
        ]
    res = run_bass_kernel_spmd(
        nc, in_maps, list(range(N_CORES)),
        trace=bool(os.environ.get("BASS_TRACE")),
    )
    LAST_RESULT = res
    out = np.concatenate([res.results[c]["out"] for c in range(N_CORES)],
                         axis=0)
    return out.reshape(B, S, H)



# revision 14
# speedup vs baseline: 1.2562x; 1.0284x over previous
"""BitNet MLP (act_quant -> ternary matmul -> relu^2 -> SubLN -> act_quant ->
ternary matmul) on 8 Trainium2 NeuronCores, data-parallel over tokens.

Math notes (exactness):
- act_quant int levels (|q| <= 127) and ternary weights {-1,0,1} are exactly
  representable in bf16, so both matmuls run on the PE in bf16 with exact
  integer arithmetic (f32 PSUM accumulation, |sums| < 2^24).
- All quantization scales are folded into per-token scalars applied to the
  final [tok, 512] output: out = i2 * beta_t with
    beta_t = clip(c_t * alpha_t * Sabs_t, 1e-5) * clip(mean|w_dn|,1e-5) / 127
  where alpha_t = (clip(max|x_t|,1e-5) * clip(mean|w_up|,1e-5) / 127)^2,
  Sabs_t = max_i |relu(ih)^2 * g|, c_t = rsqrt(var_t + 1e-6).
- Rounding uses the magic-number trick (x + 1.5*2^23 - 1.5*2^23) == RNE
  round-to-integer for |x| < 2^22, matching jnp.round (half-to-even).
- SubLN variance is recovered from the quantized intermediate:
  var = alpha^2 * sum(iu^2) * (Sabs/127)^2 / (2048 * g0^2); the
  quantization error on sum(iu^2) is ~0.1% which is far below tolerance.
  (For non-constant g an extra pass computes sum((relu^2)^2) directly.)
"""
import os
import numpy as np

import concourse.bass as bass
import concourse.tile as tile
from concourse import mybir
from concourse.bass_utils import run_bass_kernel_spmd
from concourse.masks import make_identity

# ---------------------------------------------------------------------------
# Workaround for walrus "Too many sync wait commands" on the TileContext tail
# drain: split the drain's semaphore waits across single-wait SP NOPs, then
# advance the observed clocks so the real drain needs none.
import re as _re
import bass_rust as _bass_rust


def _patched_drain_and_barrier(self, tick_clock, wait_clock):
    gc = tick_clock.global_clock
    ticks = list(map(int, _re.findall(r"\d+", repr(gc))))
    n = len(ticks)
    nonzero = [(i, t) for i, t in enumerate(ticks) if t > 0]
    for i, t in nonzero:
        sub = [0] * n
        sub[i] = t
        sub_scoped = _bass_rust.ScopedClock({None: _bass_rust.VectorClock(sub)})
        nop = self.nc.sync.nop()
        wait_clock.add_sem_waits(nop.ins, sub_scoped)
        for ec in wait_clock.engine_clocks:
            ec.update_past(sub_scoped)
    drain_inst = self.nc.sync.drain()
    wait_clock.add_sem_waits(drain_inst.ins,
                             _bass_rust.ScopedClock({None: gc}))
    self.nc.all_engine_barrier()
    popped = self.nc._tile_sem_poison_stack.pop()
    assert popped is self._sem_poison
    self.nc.clear_and_free_semaphores(list(self.sems.allocated().values()))
    self.nc.all_engine_barrier()


tile.TileContext._drain_and_barrier = _patched_drain_and_barrier


def _split_sync_waits(nc, keep_default=1):
    """walrus caps the number of semaphore waits a single instruction can
    carry (CTRL ops take only 1; compute ops a few). Hoist excess waits onto
    single-wait NOPs inserted immediately before the instruction on the same
    engine — identical semantics, engines execute in order."""
    import dataclasses
    keep_by_op = {}
    proto = None
    for f in nc.m.functions:
        for bb in f.blocks:
            for inst in bb.instructions:
                if type(inst).__name__ == "InstNoOp":
                    proto = inst
                    break
            if proto is not None:
                break
        if proto is not None:
            break
    counter = [0]
    for f in nc.m.functions:
        new_blocks = []
        for bb in f.blocks:
            out = []
            changed = False
            for inst in bb.instructions:
                si = inst.sync_info
                ow = list(si.on_wait) if si is not None and si.on_wait else []
                keep = keep_by_op.get(inst.opcode, keep_default)
                if len(ow) > keep:
                    assert proto is not None, "no NoOp prototype found yet"
                    for w in ow[:-keep]:
                        counter[0] += 1
                        nop = dataclasses.replace(
                            proto,
                            name=f"I-waitsplit-{counter[0]}",
                            engine=inst.engine,
                            sync_info=_bass_rust.SyncInfo(on_wait=[w],
                                                          on_update=[]),
                        )
                        out.append(nop)
                    si.on_wait = ow[-keep:]
                    changed = True
                out.append(inst)
            if changed:
                bb2 = _bass_rust.BasicBlock(name=bb.name, instructions=out)
                bb2.IsExit = bb.IsExit
                bb2.IsLoopEntry = bb.IsLoopEntry
                bb2.IsPredicated = bb.IsPredicated
                new_blocks.append(bb2)
            else:
                new_blocks.append(bb)
        f.blocks = new_blocks
# ---------------------------------------------------------------------------

F32 = mybir.dt.float32
BF16 = mybir.dt.bfloat16
ALU = mybir.AluOpType
AF = mybir.ActivationFunctionType

N_CORES = 8
B, S, H, I = 8, 8192, 512, 2048
TOK = B * S                  # 65536 tokens total
TPC = TOK // N_CORES         # 8192 tokens per core
P = 128                      # partition tile
NT = TPC // P                # 64 token tiles per core
NKH = H // P                 # 4 k-tiles over H
NKI = I // P                 # 16 k-tiles over I
NB = I // 512                # 4 psum banks for the up matmul

MAGIC = 12582912.0           # 1.5 * 2^23: RNE round-to-int trick
EPS = 1e-6                   # SubLN eps (from reference)

LAST_RESULT = None           # set by kernel() for test harness introspection


def _emit_weight_quant(nc, stage, junkp, ps, consts, wT_dram, n_ktiles,
                       nsub, name, magicb):
    """Quantize a (host-pre-transposed) weight matrix to ternary bf16 tiles.

    wT_dram: [n_ktiles*128, nsub*512] f32 in DRAM (contraction dim on rows).
    Returns (list of [128, nsub, 512] bf16 sbuf tiles, scale_recip [128,1],
    meanclip [128,1]) where meanclip = clip(mean|w|, 1e-5) broadcast to all
    partitions.
    """
    n_elem = n_ktiles * 128 * nsub * 512

    # pass 1: per-partition abs sums
    asum = consts.tile([P, n_ktiles], F32, tag=f"{name}_asum")
    for k in range(n_ktiles):
        wf = stage.tile([P, nsub * 512], F32, tag="stage")
        nc.gpsimd.dma_start(out=wf, in_=wT_dram[k * P:(k + 1) * P, :])
        junk = junkp.tile([P, nsub * 512], BF16, tag="junk")
        nc.scalar.activation(out=junk, in_=wf, func=AF.Abs,
                             accum_out=asum[:, k:k + 1])
    tot = consts.tile([P, 1], F32, tag=f"{name}_tot")
    nc.vector.tensor_reduce(out=tot, in_=asum, axis=mybir.AxisListType.X,
                            op=ALU.add)
    # broadcast-sum across partitions: ones128.T @ tot
    ones128 = stage.tile([P, P], F32, tag="ones128")
    nc.vector.memset(ones128, 1.0)
    totp = ps.tile([P, 1], F32, tag="totp")
    nc.tensor.matmul(out=totp, lhsT=ones128, rhs=tot, start=True, stop=True)
    gsum = consts.tile([P, 1], F32, tag=f"{name}_gsum")
    nc.scalar.copy(out=gsum, in_=totp)
    # mean -> clip -> reciprocal scale
    meanclip = consts.tile([P, 1], F32, tag=f"{name}_meanclip")
    nc.vector.tensor_scalar(out=meanclip, in0=gsum, scalar1=1.0 / n_elem,
                            scalar2=1e-5, op0=ALU.mult, op1=ALU.max)
    swq = consts.tile([P, 1], F32, tag=f"{name}_swq")
    nc.vector.reciprocal(out=swq, in_=meanclip)

    # pass 2: re-load, round+clip to ternary bf16
    wq_tiles = []
    for k in range(n_ktiles):
        wf = stage.tile([P, nsub * 512], F32, tag="stage")
        nc.gpsimd.dma_start(out=wf, in_=wT_dram[k * P:(k + 1) * P, :])
        rt = stage.tile([P, nsub * 512], F32, tag="stage_rt")
        nc.scalar.activation(out=rt, in_=wf, func=AF.Identity,
                             bias=magicb, scale=swq)
        cl = stage.tile([P, nsub * 512], F32, tag="stage_cl")
        nc.vector.tensor_scalar(out=cl, in0=rt, scalar1=MAGIC, scalar2=1.0,
                                op0=ALU.subtract, op1=ALU.min)
        wq = consts.tile([P, nsub, 512], BF16, tag=f"{name}_wq{k}")
        nc.vector.tensor_scalar(out=wq.rearrange("p a b -> p (a b)"), in0=cl,
                                scalar1=-1.0, scalar2=None, op0=ALU.max)
        wq_tiles.append(wq)
    return wq_tiles, meanclip


def build_nc(general_g: bool):
    nc = bass.Bass()
    x_d = nc.dram_tensor("x", [TPC, H], F32, kind="ExternalInput")
    wupT_d = nc.dram_tensor("wupT", [H, I], F32, kind="ExternalInput")
    wdnT_d = nc.dram_tensor("wdnT", [I, H], F32, kind="ExternalInput")
    g_d = nc.dram_tensor("g", [I], F32, kind="ExternalInput")
    out_d = nc.dram_tensor("out", [TPC, H], F32, kind="ExternalOutput")

    from contextlib import ExitStack
    with ExitStack() as ctx:
        tc = ctx.enter_context(tile.TileContext(nc))

        # ---------------- constants / weight prep ----------------
        consts = ctx.enter_context(tc.tile_pool(name="consts", bufs=1))

        ident = consts.tile([P, P], BF16)
        make_identity(nc, ident)

        magicb = consts.tile([P, 1], F32)
        nc.vector.memset(magicb, MAGIC)

        # g broadcast to all partitions: [128, I] f32
        g_bc = consts.tile([P, I], F32)
        g_ap = g_d[:]
        g_bcast_ap = bass.AP(tensor=g_ap.tensor, offset=g_ap.offset,
                             ap=[[0, P]] + list(g_ap.ap))
        nc.gpsimd.dma_start(out=g_bc, in_=g_bcast_ap)

        g0b = consts.tile([P, 1], F32)
        with tc.tile_pool(name="wstage", bufs=2) as stage, \
                tc.tile_pool(name="wjunk", bufs=2) as junkp, \
                tc.tile_pool(name="wps", bufs=1, space="PSUM") as wps:
            # g0 broadcast [128,1] via K=1 matmul with ones
            ones_row = stage.tile([1, P], F32, tag="ones_row")
            nc.vector.memset(ones_row, 1.0)
            g0_sb = stage.tile([1, 1], F32, tag="g0sb")
            nc.gpsimd.dma_start(out=g0_sb, in_=g_d[0:1])
            g0_ps = wps.tile([P, 1], F32, tag="g0ps")
            nc.tensor.matmul(out=g0_ps, lhsT=ones_row, rhs=g0_sb, start=True,
                             stop=True)
            nc.scalar.copy(out=g0b, in_=g0_ps)

            wup_q, up_meanclip = _emit_weight_quant(
                nc, stage, junkp, wps, consts, wupT_d, NKH, NB, "wup", magicb)
            wdn_q, dn_meanclip = _emit_weight_quant(
                nc, stage, junkp, wps, consts, wdnT_d, NKI, 1, "wdn", magicb)

        # k1b = clip(mean|w_up|,1e-5)/127  (per-token gamma multiplier)
        k1b = consts.tile([P, 1], F32)
        nc.vector.tensor_scalar_mul(out=k1b, in0=up_meanclip, scalar1=1.0 / 127.0)
        # wdk = clip(mean|w_dn|,1e-5)/127  (final output multiplier)
        wdk = consts.tile([P, 1], F32)
        nc.vector.tensor_scalar_mul(out=wdk, in0=dn_meanclip, scalar1=1.0 / 127.0)
        # sg127 = sign(g0)*127 (quant scale sign), g0a = |g0|
        sg127 = consts.tile([P, 1], F32)
        nc.scalar.activation(out=sg127, in_=g0b, func=AF.Sign)
        nc.vector.tensor_scalar_mul(out=sg127, in0=sg127, scalar1=127.0)
        g0a = consts.tile([P, 1], F32)
        nc.scalar.activation(out=g0a, in_=g0b, func=AF.Abs)

        # ---------------- main token-tile pipeline ----------------
        # isg = sign(g0)/127 (or 1/127 for general g): folds the quant scale
        # sign so d = recip(clip(S)*isg) = sign*127/clip(S) in 2 small ops.
        isg = consts.tile([P, 1], F32)
        if general_g:
            nc.vector.memset(isg, 1.0 / 127.0)
        else:
            nc.vector.tensor_scalar_mul(out=isg, in0=sg127,
                                        scalar1=1.0 / (127.0 * 127.0))

        BG = 8  # tiles per small-op batch
        KV = 1.0 / (127.0 * 127.0 * I)

        xs_pool = ctx.enter_context(tc.tile_pool(name="xs", bufs=2 * BG))
        xq_pool = ctx.enter_context(tc.tile_pool(name="xqp", bufs=3))
        big = ctx.enter_context(tc.tile_pool(name="big", bufs=2))
        iup = ctx.enter_context(tc.tile_pool(name="iup", bufs=3))
        outp = ctx.enter_context(tc.tile_pool(name="outp", bufs=BG + 1))
        o2p = ctx.enter_context(tc.tile_pool(name="o2p", bufs=3))
        junkp = ctx.enter_context(tc.tile_pool(name="mjunk", bufs=1))
        small = ctx.enter_context(tc.tile_pool(name="small", bufs=3))
        batchp = ctx.enter_context(tc.tile_pool(name="batchp", bufs=2))
        ps_xT = ctx.enter_context(tc.tile_pool(name="ps_xT", bufs=1,
                                               space="PSUM"))
        ps_ih = ctx.enter_context(tc.tile_pool(name="ps_ih", bufs=1,
                                               space="PSUM"))
        ps_iuT = ctx.enter_context(tc.tile_pool(name="ps_iuT", bufs=1,
                                                space="PSUM"))
        ps_o = ctx.enter_context(tc.tile_pool(name="ps_o", bufs=1,
                                              space="PSUM"))

        IH2 = I // 2  # up-matmul accumulates in two 2-bank halves

        def phase_a(ib, state):
            """DMA x tiles + per-token absmax, then batched x-scale chain."""
            xm8 = batchp.tile([P, BG], F32, tag="xm8")
            x_tiles = []
            for j in range(BG):
                r0 = (ib + j) * P
                x_sb = xs_pool.tile([P, H], F32, tag="x")
                nc.sync.dma_start(out=x_sb, in_=x_d[r0:r0 + P, :])
                x_tiles.append(x_sb)
                nc.vector.tensor_reduce(out=xm8[:, j:j + 1], in_=x_sb,
                                        axis=mybir.AxisListType.X, op=ALU.max,
                                        apply_absolute_value=True)
            t08 = batchp.tile([P, BG], F32, tag="t08")
            nc.vector.tensor_scalar_max(out=t08, in0=xm8, scalar1=1e-5)
            xr8 = batchp.tile([P, BG], F32, tag="xr8")
            nc.vector.reciprocal(out=xr8, in_=t08)
            xsc8 = batchp.tile([P, BG], F32, tag="xsc8")
            nc.vector.tensor_scalar_mul(out=xsc8, in0=xr8, scalar1=127.0)
            state[ib] = (x_tiles, t08, xsc8)

        def phase_bc(ib, state):
            x_tiles, t08, xsc8 = state.pop(ib)
            Sm8 = batchp.tile([P, BG], F32, tag="Sm8")
            q28 = batchp.tile([P, BG], F32, tag="q28")
            q2g8 = None
            if general_g:
                q2g8 = batchp.tile([P, BG], F32, tag="q2g8")
            o_tiles = []

            for j in range(BG):
                x_sb = x_tiles[j]
                # quantize x (RNE round via magic): ACT + DVE
                xq = xq_pool.tile([P, H], F32, tag="xq")
                nc.scalar.activation(out=xq, in_=x_sb, func=AF.Identity,
                                     bias=magicb, scale=xsc8[:, j:j + 1])
                ix = xq_pool.tile([P, H], BF16, tag="ix")
                nc.vector.tensor_scalar(out=ix, in0=xq, scalar1=MAGIC,
                                        scalar2=None, op0=ALU.subtract)
                # transpose ix via PE, drain on ACT
                xT_ps = ps_xT.tile([P, NKH, P], BF16, tag="xT")
                for k in range(NKH):
                    nc.tensor.transpose(out=xT_ps[:, k, :],
                                        in_=ix[:, k * P:(k + 1) * P],
                                        identity=ident)
                xT_sb = xq_pool.tile([P, NKH, P], BF16, tag="xTsb")
                nc.scalar.copy(out=xT_sb, in_=xT_ps)

                # up matmul in two halves (each 2 PSUM banks) so the next
                # tile's matmuls only wait on a half-drain
                r_sb = big.tile([P, I], F32, tag="r")
                smh = small.tile([P, 2], F32, tag="smh")
                for h in range(2):
                    ihh = ps_ih.tile([P, IH2], F32, tag="ih")
                    for nb in range(2):
                        lo = nb * 512
                        for k in range(NKH):
                            nc.tensor.matmul(
                                out=ihh[:, lo:lo + 512],
                                lhsT=xT_sb[:, k, :],
                                rhs=wup_q[k][:, 2 * h + nb, :],
                                start=(k == 0), stop=(k == NKH - 1))
                    nc.scalar.activation(out=r_sb[:, h * IH2:(h + 1) * IH2],
                                         in_=ihh, func=AF.Relu)
                    if not general_g:
                        nc.vector.tensor_reduce(out=smh[:, h:h + 1], in_=ihh,
                                                axis=mybir.AxisListType.X,
                                                op=ALU.max)

                if general_g:
                    s_sb = big.tile([P, I], F32, tag="s")
                    nc.gpsimd.tensor_tensor(out=s_sb, in0=r_sb, in1=r_sb,
                                            op=ALU.mult)
                    sq_in = big.tile([P, I], F32, tag="sg")
                    nc.vector.tensor_tensor(out=sq_in, in0=s_sb, in1=g_bc,
                                            op=ALU.mult)
                    junk3 = junkp.tile([P, I], BF16, tag="junk3")
                    nc.scalar.activation(out=junk3, in_=s_sb, func=AF.Square,
                                         accum_out=q2g8[:, j:j + 1])
                    nc.vector.tensor_reduce(out=Sm8[:, j:j + 1], in_=sq_in,
                                            axis=mybir.AxisListType.X,
                                            op=ALU.max,
                                            apply_absolute_value=True)
                    sc2 = small.tile([P, 1], F32, tag="sc2")
                    nc.vector.tensor_scalar(out=sc2, in0=Sm8[:, j:j + 1],
                                            scalar1=1e-30, scalar2=isg,
                                            op0=ALU.max, op1=ALU.mult)
                    dr = small.tile([P, 1], F32, tag="dr")
                    nc.vector.reciprocal(out=dr, in_=sc2)
                    rt = big.tile([P, I], F32, tag="rt")
                    nc.vector.tensor_scalar(out=rt, in0=sq_in, scalar1=dr,
                                            scalar2=MAGIC, op0=ALU.mult,
                                            op1=ALU.add)
                    iu = iup.tile([P, I], BF16, tag="iu")
                    nc.vector.tensor_scalar(out=iu, in0=rt, scalar1=MAGIC,
                                            scalar2=None, op0=ALU.subtract)
                else:
                    # s' = r*r on GPSIMD
                    s_sb = big.tile([P, I], F32, tag="s")
                    nc.gpsimd.tensor_tensor(out=s_sb, in0=r_sb, in1=r_sb,
                                            op=ALU.mult)
                    # rmax = max over both halves (straight from PSUM above)
                    nc.vector.tensor_reduce(out=Sm8[:, j:j + 1], in_=smh,
                                            axis=mybir.AxisListType.X,
                                            op=ALU.max)
                    mr = small.tile([P, 1], F32, tag="mr")
                    nc.vector.tensor_scalar_max(out=mr, in0=Sm8[:, j:j + 1],
                                                scalar1=1e-15)
                    sc2 = small.tile([P, 1], F32, tag="sc2")
                    nc.vector.tensor_scalar(out=sc2, in0=mr, scalar1=mr,
                                            scalar2=isg, op0=ALU.mult,
                                            op1=ALU.mult)
                    dr = small.tile([P, 1], F32, tag="dr")
                    nc.vector.reciprocal(out=dr, in_=sc2)
                    rt = big.tile([P, I], F32, tag="rt")
                    nc.vector.tensor_scalar(out=rt, in0=s_sb, scalar1=dr,
                                            scalar2=MAGIC, op0=ALU.mult,
                                            op1=ALU.add)
                    iu = iup.tile([P, I], BF16, tag="iu")
                    nc.vector.tensor_scalar(out=iu, in0=rt, scalar1=MAGIC,
                                            scalar2=None, op0=ALU.subtract)

                # q2 = sum(iu^2) (ACT square + accumulate)
                junk2 = junkp.tile([P, I], BF16, tag="junk2")
                nc.scalar.activation(out=junk2, in_=iu, func=AF.Square,
                                     accum_out=q28[:, j:j + 1])

                # transpose iu via PE, drain on ACT
                iuT_ps = ps_iuT.tile([P, NKI, P], BF16, tag="iuT")
                for k in range(NKI):
                    nc.tensor.transpose(out=iuT_ps[:, k, :],
                                        in_=iu[:, k * P:(k + 1) * P],
                                        identity=ident)
                iuT_sb = iup.tile([P, NKI, P], BF16, tag="iuTsb")
                nc.scalar.copy(out=iuT_sb, in_=iuT_ps)

                # down matmul + plain drain (beta applied later, batched)
                o_ps = ps_o.tile([P, H], F32, tag="o")
                for k in range(NKI):
                    nc.tensor.matmul(out=o_ps, lhsT=iuT_sb[:, k, :],
                                     rhs=wdn_q[k][:, 0, :],
                                     start=(k == 0), stop=(k == NKI - 1))
                o_sb = outp.tile([P, H], F32, tag="osb")
                nc.scalar.copy(out=o_sb, in_=o_ps)
                o_tiles.append(o_sb)

            # --- batched beta chain ---
            scc8 = batchp.tile([P, BG], F32, tag="scc8")
            if general_g:
                nc.vector.tensor_scalar_max(out=scc8, in0=Sm8, scalar1=1e-30)
            else:
                ra8 = batchp.tile([P, BG], F32, tag="ra8")
                nc.vector.tensor_scalar_max(out=ra8, in0=Sm8, scalar1=0.0)
                ssq8 = batchp.tile([P, BG], F32, tag="ssq8")
                nc.vector.tensor_tensor(out=ssq8, in0=ra8, in1=ra8,
                                        op=ALU.mult)
                nc.vector.tensor_scalar_max(out=scc8, in0=ssq8,
                                            scalar1=1e-30)
            ga8 = batchp.tile([P, BG], F32, tag="ga8")
            nc.vector.tensor_scalar_mul(out=ga8, in0=t08, scalar1=k1b)
            al8 = batchp.tile([P, BG], F32, tag="al8")
            nc.vector.tensor_tensor(out=al8, in0=ga8, in1=ga8, op=ALU.mult)
            m18 = batchp.tile([P, BG], F32, tag="m18")
            nc.vector.tensor_tensor(out=m18, in0=al8, in1=scc8, op=ALU.mult)
            v18 = batchp.tile([P, BG], F32, tag="v18")
            Ve8 = batchp.tile([P, BG], F32, tag="Ve8")
            if general_g:
                al28 = batchp.tile([P, BG], F32, tag="al28")
                nc.vector.tensor_tensor(out=al28, in0=al8, in1=al8,
                                        op=ALU.mult)
                nc.vector.tensor_tensor(out=v18, in0=al28, in1=q2g8,
                                        op=ALU.mult)
                nc.vector.tensor_scalar(out=Ve8, in0=v18, scalar1=1.0 / I,
                                        scalar2=EPS, op0=ALU.mult,
                                        op1=ALU.add)
            else:
                m28 = batchp.tile([P, BG], F32, tag="m28")
                nc.vector.tensor_tensor(out=m28, in0=m18, in1=m18,
                                        op=ALU.mult)
                nc.vector.tensor_tensor(out=v18, in0=m28, in1=q28,
                                        op=ALU.mult)
                nc.vector.tensor_scalar(out=Ve8, in0=v18, scalar1=KV,
                                        scalar2=EPS, op0=ALU.mult,
                                        op1=ALU.add)
            sq8 = batchp.tile([P, BG], F32, tag="sq8")
            nc.scalar.activation(out=sq8, in_=Ve8, func=AF.Sqrt)
            cr8 = batchp.tile([P, BG], F32, tag="cr8")
            nc.vector.reciprocal(out=cr8, in_=sq8)
            h18 = batchp.tile([P, BG], F32, tag="h18")
            nc.vector.tensor_tensor(out=h18, in0=cr8, in1=cr8, op=ALU.mult)
            h28 = batchp.tile([P, BG], F32, tag="h28")
            nc.vector.tensor_tensor(out=h28, in0=h18, in1=Ve8, op=ALU.mult)
            h38 = batchp.tile([P, BG], F32, tag="h38")
            nc.vector.tensor_scalar(out=h38, in0=h28, scalar1=-0.5,
                                    scalar2=1.5, op0=ALU.mult, op1=ALU.add)
            c8 = batchp.tile([P, BG], F32, tag="c8")
            nc.vector.tensor_tensor(out=c8, in0=cr8, in1=h38, op=ALU.mult)
            if general_g:
                m1g8 = m18
            else:
                m1g8 = batchp.tile([P, BG], F32, tag="m1g8")
                nc.vector.tensor_scalar_mul(out=m1g8, in0=m18, scalar1=g0a)
            mu8 = batchp.tile([P, BG], F32, tag="mu8")
            nc.vector.tensor_tensor(out=mu8, in0=c8, in1=m1g8, op=ALU.mult)
            b8 = batchp.tile([P, BG], F32, tag="b8")
            nc.vector.tensor_scalar(out=b8, in0=mu8, scalar1=1e-5,
                                    scalar2=wdk, op0=ALU.max, op1=ALU.mult)

            # --- scale + store ---
            for j in range(BG):
                r0 = (ib + j) * P
                o2 = o2p.tile([P, H], F32, tag="o2")
                nc.vector.tensor_scalar_mul(out=o2, in0=o_tiles[j],
                                            scalar1=b8[:, j:j + 1])
                nc.sync.dma_start(out=out_d[r0:r0 + P, :], in_=o2)

        # software-pipelined emission: batch ib+1's loads are issued before
        # batch ib's compute so DMA/absmax overlap the previous batch
        state = {}
        phase_a(0, state)
        for ib in range(0, NT, BG):
            if ib + BG < NT:
                phase_a(ib + BG, state)
            phase_bc(ib, state)

    _split_sync_waits(nc)
    return nc


def build_fast(g0: float):
    """Const-g fast path.

    Per 128-token tile (exact integer math, scales folded into final beta):
      DVE : xq = x*sc + MAGIC           (round-to-int via magic, f32)
      ACT : ix = xq - MAGIC -> bf16     (exact int8 levels)
      PE  : xT = transpose(ix)          (4x N=128)
      PE  : h  = ixT.T @ wup_q          (16x N=512 bf16, exact ints in PSUM)
      DVE : s2 = max(h,0)*h             (= relu(h)^2, one STT from PSUM)
      DVE : S2m = max(s2)               (= Rm^2, feeds d = 127/Rm^2)
      ACT : t1 = s2*d + MAGIC           (per-token scale via ACT scale port)
      DVE : iu = t1 - MAGIC -> bf16     (exact int levels 0..127)
      GPS : q2 = sum(iu^2)              (STT with accum, junk main output)
      PE  : iuT = transpose(iu)         (16x N=128), ACT drains
      PE  : o  = iuT.T @ wdn_q          (16x N=512)
      ACT : o_sb = copy(o)
      DVE : out = o_sb * beta, DMA out  (beta via batched per-8 chain)
    Down-matmuls are emitted one 4-tile group behind the up-matmuls so the
    PE never waits on the s2->iu chain.
    """
    nc = bass.Bass()
    x_d = nc.dram_tensor("x", [TPC, H], F32, kind="ExternalInput")
    wupT_d = nc.dram_tensor("wupT", [H, I], F32, kind="ExternalInput")
    wdnT_d = nc.dram_tensor("wdnT", [I, H], F32, kind="ExternalInput")
    out_d = nc.dram_tensor("out", [TPC, H], F32, kind="ExternalOutput")

    BG = 8          # stats/beta batch
    GRP = 4         # pipeline group (down-matmul lag)
    IH2 = I // 2
    KV = 1.0 / (127.0 * 127.0 * I)
    g0a = abs(g0)
    g0s = 1.0 if g0 >= 0 else -1.0

    from contextlib import ExitStack
    with ExitStack() as ctx:
        tc = ctx.enter_context(tile.TileContext(nc))

        consts = ctx.enter_context(tc.tile_pool(name="consts", bufs=1))
        ident = consts.tile([P, P], BF16)
        make_identity(nc, ident)
        identf = consts.tile([P, P], F32)
        make_identity(nc, identf)
        wup_q = consts.tile([P, NKH, I], BF16)
        wdn_q = consts.tile([P, NKI, H], BF16)
        k1b = consts.tile([P, 1], F32)
        wdk = consts.tile([P, 1], F32)
        magicb = consts.tile([P, 1], F32)
        nc.vector.memset(magicb, MAGIC)
        nmagicb = consts.tile([P, 1], F32)
        nc.vector.memset(nmagicb, -MAGIC)

        # pools that must exist before weight prep so x loads / absmax /
        # quant / transposes overlap the prologue
        xs_pool = ctx.enter_context(tc.tile_pool(name="xs", bufs=9))
        batchp = ctx.enter_context(tc.tile_pool(name="batchp", bufs=3))
        xq_pool = ctx.enter_context(tc.tile_pool(name="xqp", bufs=3))
        ix_pool = ctx.enter_context(tc.tile_pool(name="ixp", bufs=3))
        xT_pool = ctx.enter_context(tc.tile_pool(name="xTp", bufs=12))
        ps_tp = ctx.enter_context(tc.tile_pool(name="ps_tp", bufs=2,
                                               space="PSUM"))

        state = {}

        def load_batch(ib):
            xm8 = batchp.tile([P, BG], F32, tag="xm8")
            x_tiles = []
            for jj in range(BG):
                r0 = (ib + jj) * P
                x_sb = xs_pool.tile([P, H], F32, tag="x")
                nc.sync.dma_start(out=x_sb, in_=x_d[r0:r0 + P, :])
                nc.vector.tensor_reduce(out=xm8[:, jj:jj + 1], in_=x_sb,
                                        axis=mybir.AxisListType.X, op=ALU.max,
                                        apply_absolute_value=True)
                x_tiles.append(x_sb)
            t08 = batchp.tile([P, BG], F32, tag="t08")
            nc.vector.tensor_scalar_max(out=t08, in0=xm8, scalar1=1e-5)
            xr8 = batchp.tile([P, BG], F32, tag="xr8")
            nc.vector.reciprocal(out=xr8, in_=t08)
            xsc8 = batchp.tile([P, BG], F32, tag="xsc8")
            nc.vector.tensor_scalar_mul(out=xsc8, in0=xr8, scalar1=127.0)
            S2m8 = batchp.tile([P, BG], F32, tag="S2m8")
            q28 = batchp.tile([P, BG], F32, tag="q28")
            state[ib] = dict(x=x_tiles, t08=t08, xsc8=xsc8, S2m8=S2m8,
                             q28=q28)

        load_batch(0)
        load_batch(BG)
        tstate = {}

        def front_x(j):
            ib = (j // BG) * BG
            jj = j - ib
            st = state[ib]
            x_sb = st["x"][jj]
            xq = xq_pool.tile([P, H], F32, tag="xq")
            nc.vector.tensor_scalar(out=xq, in0=x_sb,
                                    scalar1=st["xsc8"][:, jj:jj + 1],
                                    scalar2=MAGIC, op0=ALU.mult, op1=ALU.add)
            ixt = ix_pool.tile([P, H], BF16, tag="ix")
            nc.scalar.activation(out=ixt, in_=xq, func=AF.Identity,
                                 bias=nmagicb)
            tp = ps_tp.tile([P, 8, P], BF16, tag="tp")
            for k in range(NKH):
                nc.tensor.transpose(out=tp[:, k, :],
                                    in_=ixt[:, k * P:(k + 1) * P],
                                    identity=ident)
            xT = xT_pool.tile([P, NKH, P], BF16, tag="xT")
            nc.scalar.copy(out=xT, in_=tp[:, :NKH, :])
            tstate[j] = dict(xT=xT)

        for _j in range(2 * GRP):
            front_x(_j)

        # ---------------- weight prep (single load) ----------------
        with tc.tile_pool(name="wstage", bufs=1) as wst, \
                tc.tile_pool(name="wscr", bufs=2) as wsc, \
                tc.tile_pool(name="wjunk", bufs=2) as wjk, \
                tc.tile_pool(name="wps", bufs=1, space="PSUM") as wps:
            up_st = wst.tile([P, NKH, I], F32, tag="upst")
            dn_st = wst.tile([P, NKI, H], F32, tag="dnst")
            for k in range(NKH):
                nc.gpsimd.dma_start(out=up_st[:, k, :],
                                    in_=wupT_d[k * P:(k + 1) * P, :])

            asum_u = consts.tile([P, NKH], F32)
            asum_d = consts.tile([P, NKH], F32)
            for k in range(NKH):
                jku = wjk.tile([P, I], BF16, tag="jk")
                nc.scalar.activation(out=jku, in_=up_st[:, k, :], func=AF.Abs,
                                     accum_out=asum_u[:, k:k + 1])
            ones128 = wsc.tile([P, P], F32, tag="ones", bufs=1)
            nc.vector.memset(ones128, 1.0)

            def total_meanclip(asum, n_elem, mc_out):
                tot = consts.tile([P, 1], F32)
                nc.vector.tensor_reduce(out=tot, in_=asum,
                                        axis=mybir.AxisListType.X, op=ALU.add)
                totp = wps.tile([P, 1], F32, tag="totp")
                nc.tensor.matmul(out=totp, lhsT=ones128, rhs=tot, start=True,
                                 stop=True)
                gsum = consts.tile([P, 1], F32)
                nc.scalar.copy(out=gsum, in_=totp)
                nc.vector.tensor_scalar(out=mc_out, in0=gsum,
                                        scalar1=1.0 / n_elem, scalar2=1e-5,
                                        op0=ALU.mult, op1=ALU.max)

            mc_u = consts.tile([P, 1], F32)
            mc_d = consts.tile([P, 1], F32)
            total_meanclip(asum_u, H * I, mc_u)
            swq_u = consts.tile([P, 1], F32)
            nc.vector.reciprocal(out=swq_u, in_=mc_u)
            nc.vector.tensor_scalar_mul(out=k1b, in0=mc_u, scalar1=1.0 / 127.0)

            def quant_chunk(src, dst, swq):
                qt = wsc.tile(list(src.shape), F32, tag="qt")
                nc.scalar.activation(out=qt, in_=src, func=AF.Identity,
                                     bias=magicb, scale=swq)
                qu = wsc.tile(list(src.shape), F32, tag="qu")
                nc.vector.tensor_scalar(out=qu, in0=qt, scalar1=MAGIC,
                                        scalar2=1.0, op0=ALU.subtract,
                                        op1=ALU.min)
                nc.vector.tensor_scalar(out=dst, in0=qu, scalar1=-1.0,
                                        scalar2=None, op0=ALU.max)

            for k in range(NKH):
                quant_chunk(up_st[:, k, :], wup_q[:, k, :], swq_u)

            # --- wdn prep after wup so the first up-matmuls start earlier
            for k in range(NKI):
                nc.gpsimd.dma_start(out=dn_st[:, k, :],
                                    in_=wdnT_d[k * P:(k + 1) * P, :])
            for k in range(NKH):
                jkd = wjk.tile([P, NKH, H], BF16, tag="jkd")
                nc.scalar.activation(out=jkd,
                                     in_=dn_st[:, k * NKH:(k + 1) * NKH, :],
                                     func=AF.Abs,
                                     accum_out=asum_d[:, k:k + 1])
            total_meanclip(asum_d, H * I, mc_d)
            swq_d = consts.tile([P, 1], F32)
            nc.vector.reciprocal(out=swq_d, in_=mc_d)
            nc.vector.tensor_scalar_mul(out=wdk, in0=mc_d, scalar1=1.0 / 127.0)
            for k in range(NKH):
                quant_chunk(dn_st[:, k * NKH:(k + 1) * NKH, :],
                            wdn_q[:, k * NKH:(k + 1) * NKH, :], swq_d)

        # ---------------- main-loop pools ----------------
        r_pool = ctx.enter_context(tc.tile_pool(name="rp", bufs=3))
        s2_pool = ctx.enter_context(tc.tile_pool(name="s2p", bufs=2))
        t1_pool = ctx.enter_context(tc.tile_pool(name="t1p", bufs=2))
        iu_pool = ctx.enter_context(tc.tile_pool(name="iup", bufs=8))
        iuT_pool = ctx.enter_context(tc.tile_pool(name="iuTp", bufs=5))
        dj_pool = ctx.enter_context(tc.tile_pool(name="djp", bufs=2))
        o_pool = ctx.enter_context(tc.tile_pool(name="op", bufs=8))
        o2_pool = ctx.enter_context(tc.tile_pool(name="o2p", bufs=4))
        d_pool = ctx.enter_context(tc.tile_pool(name="dp", bufs=2))
        bb_pool = ctx.enter_context(tc.tile_pool(name="bbp", bufs=2))
        ps_h = ctx.enter_context(tc.tile_pool(name="ps_h", bufs=2,
                                              space="PSUM"))
        ps_o = ctx.enter_context(tc.tile_pool(name="ps_o", bufs=2,
                                              space="PSUM"))

        def front_mm(j):
            ib = (j // BG) * BG
            jj = j - ib
            st = state[ib]
            xT = tstate[j].pop("xT")
            r = r_pool.tile([P, I], F32, tag="r")
            for half in range(2):
                hh = ps_h.tile([P, IH2], F32, tag="h")
                for nb in range(2):
                    lo = nb * 512
                    for k in range(NKH):
                        nc.tensor.matmul(
                            out=hh[:, lo:lo + 512],
                            lhsT=xT[:, k, :],
                            rhs=wup_q[:, k, (2 * half + nb) * 512:
                                      (2 * half + nb + 1) * 512],
                            start=(k == 0), stop=(k == NKH - 1))
                nc.scalar.activation(out=r[:, half * IH2:(half + 1) * IH2],
                                     in_=hh, func=AF.Relu)
            # per-token Rm = max(relu(h)) (>= 0 since r >= 0)
            nc.vector.tensor_reduce(out=st["S2m8"][:, jj:jj + 1], in_=r,
                                    axis=mybir.AxisListType.X, op=ALU.max)
            # s2 = relu(h)^2 off the critical DVE/ACT paths
            s2 = s2_pool.tile([P, I], F32, tag="s2")
            nc.gpsimd.tensor_tensor(out=s2, in0=r, in1=r, op=ALU.mult)
            tstate[j]["s2"] = s2

        def dbatch(b):
            ib = (b // BG) * BG
            jj0 = b - ib
            S2m8 = state[ib]["S2m8"]
            mr4 = d_pool.tile([P, GRP], F32, tag="mr4")
            nc.vector.tensor_scalar_max(out=mr4, in0=S2m8[:, jj0:jj0 + GRP],
                                        scalar1=1e-15)
            sc4 = d_pool.tile([P, GRP], F32, tag="sc4")
            nc.vector.tensor_tensor(out=sc4, in0=mr4, in1=mr4, op=ALU.mult)
            sc4b = d_pool.tile([P, GRP], F32, tag="sc4b")
            nc.vector.tensor_scalar_mul(out=sc4b, in0=sc4,
                                        scalar1=1.0 / 127.0)
            d4 = d_pool.tile([P, GRP], F32, tag="d4")
            nc.vector.reciprocal(out=d4, in_=sc4b)
            for j in range(b, b + GRP):
                tstate[j]["d"] = d4[:, j - b:j - b + 1]

        def quant_a(j):
            ts = tstate[j]
            t1 = t1_pool.tile([P, I], F32, tag="t1")
            nc.scalar.activation(out=t1, in_=ts["s2"], func=AF.Identity,
                                 bias=magicb, scale=ts["d"])
            iu = iu_pool.tile([P, I], BF16, tag="iu")
            nc.vector.tensor_scalar(out=iu, in0=t1, scalar1=MAGIC,
                                    scalar2=None, op0=ALU.subtract)
            ts["iu"] = iu
            del ts["s2"]

        def quant_b(j):
            ts = tstate[j]
            iu = ts.pop("iu")
            iuT = iuT_pool.tile([P, NKI, P], BF16, tag="iuT")
            for half in range(2):
                tp2 = ps_tp.tile([P, 8, P], BF16, tag="tp")
                for c in range(8):
                    kk = half * 8 + c
                    nc.tensor.transpose(out=tp2[:, c, :],
                                        in_=iu[:, kk * P:(kk + 1) * P],
                                        identity=ident)
                nc.scalar.copy(out=iuT[:, half * 8:(half + 1) * 8, :],
                               in_=tp2)
            ts["iuT"] = iuT

        def down(j):
            ib = (j // BG) * BG
            jj = j - ib
            ts = tstate[j]
            o_ps = ps_o.tile([P, H], F32, tag="o")
            for k in range(NKI):
                nc.tensor.matmul(out=o_ps, lhsT=ts["iuT"][:, k, :],
                                 rhs=wdn_q[:, k, :],
                                 start=(k == 0), stop=(k == NKI - 1))
            o_sb = o_pool.tile([P, H], F32, tag="o_sb")
            nc.scalar.copy(out=o_sb, in_=o_ps)
            # q2 = sum(iu^2) via the diagonal of iuT.T @ iuT on the PE
            dg_ps = ps_tp.tile([P, P], F32, tag="tp")
            for k in range(NKI):
                nc.tensor.matmul(out=dg_ps, lhsT=ts["iuT"][:, k, :],
                                 rhs=ts["iuT"][:, k, :],
                                 start=(k == 0), stop=(k == NKI - 1))
            dj = dj_pool.tile([P, P], F32, tag="dj")
            nc.vector.tensor_tensor(out=dj, in0=dg_ps, in1=identf,
                                    op=ALU.mult)
            nc.vector.tensor_reduce(out=state[ib]["q28"][:, jj:jj + 1],
                                    in_=dj, axis=mybir.AxisListType.X,
                                    op=ALU.add)
            ts["o"] = o_sb
            del ts["iuT"]

        def bbatch(ib):
            st = state[ib]
            ga8 = bb_pool.tile([P, BG], F32, tag="ga8")
            nc.vector.tensor_scalar_mul(out=ga8, in0=st["t08"], scalar1=k1b)
            al8 = bb_pool.tile([P, BG], F32, tag="al8")
            nc.vector.tensor_tensor(out=al8, in0=ga8, in1=ga8, op=ALU.mult)
            ssq8 = bb_pool.tile([P, BG], F32, tag="ssq8")
            nc.vector.tensor_tensor(out=ssq8, in0=st["S2m8"], in1=st["S2m8"],
                                    op=ALU.mult)
            scc8 = bb_pool.tile([P, BG], F32, tag="scc8")
            nc.vector.tensor_scalar_max(out=scc8, in0=ssq8, scalar1=1e-30)
            m18 = bb_pool.tile([P, BG], F32, tag="m18")
            nc.vector.tensor_tensor(out=m18, in0=al8, in1=scc8, op=ALU.mult)
            m28 = bb_pool.tile([P, BG], F32, tag="m28")
            nc.vector.tensor_tensor(out=m28, in0=m18, in1=m18, op=ALU.mult)
            v18 = bb_pool.tile([P, BG], F32, tag="v18")
            nc.vector.tensor_tensor(out=v18, in0=m28, in1=st["q28"],
                                    op=ALU.mult)
            Ve8 = bb_pool.tile([P, BG], F32, tag="Ve8")
            nc.vector.tensor_scalar(out=Ve8, in0=v18, scalar1=KV,
                                    scalar2=EPS, op0=ALU.mult, op1=ALU.add)
            sq8 = bb_pool.tile([P, BG], F32, tag="sq8")
            nc.scalar.activation(out=sq8, in_=Ve8, func=AF.Sqrt)
            cr8 = bb_pool.tile([P, BG], F32, tag="cr8")
            nc.vector.reciprocal(out=cr8, in_=sq8)
            h18 = bb_pool.tile([P, BG], F32, tag="h18")
            nc.vector.tensor_tensor(out=h18, in0=cr8, in1=cr8, op=ALU.mult)
            h28 = bb_pool.tile([P, BG], F32, tag="h28")
            nc.vector.tensor_tensor(out=h28, in0=h18, in1=Ve8, op=ALU.mult)
            h38 = bb_pool.tile([P, BG], F32, tag="h38")
            nc.vector.tensor_scalar(out=h38, in0=h28, scalar1=-0.5,
                                    scalar2=1.5, op0=ALU.mult, op1=ALU.add)
            c8 = bb_pool.tile([P, BG], F32, tag="c8")
            nc.vector.tensor_tensor(out=c8, in0=cr8, in1=h38, op=ALU.mult)
            mu8 = bb_pool.tile([P, BG], F32, tag="mu8")
            nc.vector.tensor_tensor(out=mu8, in0=c8, in1=m18, op=ALU.mult)
            mc8 = bb_pool.tile([P, BG], F32, tag="mc8")
            nc.vector.tensor_scalar(out=mc8, in0=mu8, scalar1=g0a,
                                    scalar2=1e-5, op0=ALU.mult, op1=ALU.max)
            b8 = bb_pool.tile([P, BG], F32, tag="b8")
            nc.vector.tensor_scalar(out=b8, in0=mc8, scalar1=wdk,
                                    scalar2=g0s, op0=ALU.mult, op1=ALU.mult)
            st["b8"] = b8

        def outt(j):
            ib = (j // BG) * BG
            jj = j - ib
            ts = tstate.pop(j)
            b8 = state[ib]["b8"]
            o2 = o2_pool.tile([P, H], F32, tag="o2")
            nc.vector.tensor_scalar_mul(out=o2, in0=ts["o"],
                                        scalar1=b8[:, jj:jj + 1])
            nc.sync.dma_start(out=out_d[j * P:(j + 1) * P, :], in_=o2)

        for b in range(0, NT, GRP):
            if b % BG == 0 and b >= BG and b + BG < NT:
                load_batch(b + BG)
            if b + 2 * GRP <= NT - GRP + 3:
                for j in range(b + 2 * GRP, min(b + 3 * GRP, NT)):
                    front_x(j)
            for j in range(b, b + GRP):
                front_mm(j)
            dbatch(b)
            if b >= GRP:
                for j in range(b - GRP, b):
                    quant_b(j)
                for j in range(b - GRP, b):
                    down(j)
            if b % BG == 0 and b >= 2 * GRP:
                bbatch(b - 2 * GRP)
            if b >= 2 * GRP:
                for j in range(b - 2 * GRP, b - GRP):
                    outt(j)
            for j in range(b, b + GRP):
                quant_a(j)
        for j in range(NT - GRP, NT):
            quant_b(j)
        for j in range(NT - GRP, NT):
            down(j)
        bbatch(NT - 2 * GRP)
        for j in range(NT - 2 * GRP, NT):
            outt(j)

    _split_sync_waits(nc)
    return nc


_NC_CACHE = {}


def kernel(x, w_up, w_down, g):
    global LAST_RESULT
    x = np.ascontiguousarray(x, dtype=np.float32)
    w_up = np.ascontiguousarray(w_up, dtype=np.float32)
    w_down = np.ascontiguousarray(w_down, dtype=np.float32)
    g = np.ascontiguousarray(g, dtype=np.float32)

    if abs(float(g[0])) < 1e-30 and np.all(g == g[0]):
        return np.zeros_like(x)

    general = not bool(np.all(g == g[0]))
    xt = x.reshape(TOK, H)
    wupT = np.ascontiguousarray(w_up.T)    # [H, I]
    wdnT = np.ascontiguousarray(w_down.T)  # [I, H]
    if general:
        key = "gen"
        if key not in _NC_CACHE:
            _NC_CACHE[key] = build_nc(True)
        nc = _NC_CACHE[key]
        in_maps = [
            {"x": xt[c * TPC:(c + 1) * TPC], "wupT": wupT, "wdnT": wdnT,
             "g": g}
            for c in range(N_CORES)
        ]
    else:
        g0 = float(g[0])
        key = ("fast", g0)
        if key not in _NC_CACHE:
            _NC_CACHE[key] = build_fast(g0)
        nc = _NC_CACHE[key]
        in_maps = [
            {"x": xt[c * TPC:(c + 1) * TPC], "wupT": wupT, "wdnT": wdnT}
            for c in range(N_CORES)
        ]
    res = run_bass_kernel_spmd(
        nc, in_maps, list(range(N_CORES)),
        trace=bool(os.environ.get("BASS_TRACE")),
    )
    LAST_RESULT = res
    out = np.concatenate([res.results[c]["out"] for c in range(N_CORES)],
                         axis=0)
    return out.reshape(B, S, H)

